# revision 49
# baseline (speedup 1.0000x reference)
"""Trainium2 Bass kernel for nn_CausalSelfAttention (B=2, N=2048, D=1024, H=16).

Sharding (8 cores): batch (2-way) x head-group (4-way, 4 heads per core).
Reference swaps K/Q roles: scores = K @ Q^T, softmax over the Q index.

v3 design (all-bf16 compute; fp8 measurably exceeds the 2e-2 gate on the
K/Q path because softmax averaging does not suppress relative weight
jitter):
- Host pre-transposes x and W_proj (no on-device DMA transposes); all x
  quarters stream in up-front while compute proceeds.
- Attention starts right after quarter-0 KQV: strips 0-1 only need the
  first n/m quarter, so the remaining KQV quarters drip into the strip
  pipeline's stall points as fillers (ACT exp and PE matmuls stay
  co-busy from ~4us on).
- ACT runs exp only; K/Q/V bias adds + casts run on DVE straight from
  PSUM (tensor_scalar / tensor_tensor).
- Softmax normalize: DVE reciprocal of the PV ones-row, gpsimd
  partition_broadcast to 64 partitions (SBUF), then one DVE
  tensor_tensor mult PSUM x SBUF -> saT bf16. No PE broadcast matmul.
- AllToAll ranges [1024, 512, 512] fire after strips 3/5/7; the 4-D
  bounce-buffer APs keep the per-shard layout explicit. Projection
  matmuls for ranges 0-1 execute inside the last exchange's window so
  only the 512-row projection trails it.
"""

import sys

import numpy as np

if "/opt/trn_rl_repo" not in sys.path:
    sys.path.insert(0, "/opt/trn_rl_repo")

import ml_dtypes
import concourse.bass as bass
import concourse.mybir as mybir
import concourse.tile as tile
from concourse import bacc
from concourse.bass_utils import run_bass_kernel_spmd

F32 = mybir.dt.float32
BF16 = mybir.dt.bfloat16
BF16_NP = ml_dtypes.bfloat16

P = 128
N = 2048          # sequence length
D = 1024          # model dim
H = 16            # total heads
HPC = 4           # heads per core
HD = 64           # head dim
DC = D // P       # 8 d-chunks
NB = 256          # attention n-block (free dim of S^T tiles)
NBLK = N // NB    # 8 strips
MB = N // P       # 16 m-blocks
CHUNK = 4         # m-blocks per PSUM strip (4*256 fp32 = 2 PSUM banks)
N_CORES = 8

RANGES = [1536, 512]
RBASE = [0, 1536]
RSUB = [r // 8 for r in RANGES]          # rows per core per range
OBASE = [0, 384]                         # output row base per range (per core)

GROUP8 = [[0, 1, 2, 3, 4, 5, 6, 7]]


def _mask_np():
    # causal mask for the diagonal m-block pair of each strip:
    # cols 0:256   (m_blk 2J,   m = 256J + p, n-cols 0:256)   keep j >= p
    # cols 256:384 (m_blk 2J+1, m = 256J+128+p, n-cols 128:256) keep j >= p
    # (the S matmul for block 2J+1 only computes the upper n-half)
    p = np.arange(P)[:, None]
    m0 = (np.arange(256)[None, :] >= p).astype(np.float32)
    m1 = (np.arange(128)[None, :] >= p).astype(np.float32)
    return np.concatenate([m0, m1], axis=1).astype(BF16_NP)


def build_kernel(tc: tile.TileContext, ctx):
    nc = tc.nc

    xtb_ext = nc.dram_tensor("xtb", [P, DC, N], BF16, kind="ExternalInput")
    wk2_ext = nc.dram_tensor("wk2", [P, 2, DC, P], BF16, kind="ExternalInput")
    wq2_ext = nc.dram_tensor("wq2", [P, 2, DC, P], BF16, kind="ExternalInput")
    wv_ext = nc.dram_tensor("wv", [P, DC, HPC * HD], BF16, kind="ExternalInput")
    bkq_ext = nc.dram_tensor("bkq", [P, 2, 2], F32, kind="ExternalInput")
    vbias_ext = nc.dram_tensor("vbias", [P, HPC * HD], F32, kind="ExternalInput")
    bproj_ext = nc.dram_tensor("bproj", [P, D], F32, kind="ExternalInput")
    wpt_ext = nc.dram_tensor("wpt", [P, DC, D], BF16, kind="ExternalInput")
    out_ext = nc.dram_tensor("out", [512, D], F32, kind="ExternalOutput")

    out = out_ext[:]

    mask_dram = nc.inline_tensor(_mask_np(), name="mask_c")

    dram = ctx.enter_context(tc.tile_pool(name="dram", bufs=1, space="DRAM"))
    const = ctx.enter_context(tc.tile_pool(name="const", bufs=1))

    # AllToAll buffers: [8 chunks (receiver), 2 t, 128 p, sub r] bf16.
    # Row-padded so the per-receiver leading dim stays explicit in the
    # lowered access pattern (strided slices don't collapse).
    CPAD = 0
    cc_in_f = [dram.tile([8, 2, P, RSUB[k] + CPAD], BF16, name=f"cc_in{k}")
               for k in range(len(RANGES))]
    cc_out_f = [dram.tile([8, 2, P, RSUB[k] + CPAD], BF16, name=f"cc_out{k}")
                for k in range(len(RANGES))]
    cc_in = [t[:, :, :, 0:RSUB[k]] for k, t in enumerate(cc_in_f)]
    cc_out = [t[:, :, :, 0:RSUB[k]] for k, t in enumerate(cc_out_f)]

    # ---------------- SBUF constants / weights ----------------
    # per-quarter x^T tiles: separate tiles keep the dependency tracker's
    # byte-range bounding boxes quarter-precise (a unified tile serializes
    # the whole pipeline behind the last x quarter's DMA)
    xtb_q = [const.tile([P, DC, N // 4], BF16, name=f"xtb{q}")
             for q in range(4)]
    wk2 = const.tile([P, 2, DC, P], BF16, name="wk2")
    wq2 = const.tile([P, 2, DC, P], BF16, name="wq2")
    wv = const.tile([P, DC, HPC * HD], BF16, name="wv")
    bkq = const.tile([P, 2, 2], F32, name="bkq")
    vbias = const.tile([P, HPC * HD], F32, name="vbias")
    bproj = const.tile([P, D], F32, name="bproj")
    mask = const.tile([P, 384], BF16, name="mask")
    wpt = const.tile([P, DC, D], BF16, name="wpt")

    # PE p-state warmup dummies (memset-initialized, no DMA dependency)
    dw = const.tile([1, 1], BF16, name="dw")
    dx = const.tile([1, 512], BF16, name="dx")

    # ---------------- KQV / attention state ----------------
    # k2/q2: [64*h2+e, pr, n] bf16 (packed head-pair partition layout)
    k2 = const.tile([P, 2, N], BF16, name="k2")
    q2 = const.tile([P, 2, N], BF16, name="q2")
    v = const.tile([P, MB, HPC * (HD + 1)], BF16, name="v")
    saT = const.tile([P, 2, N], BF16, name="saT")

    NQ = N // 4

    def emit_x_load(ns, split=False):
        qsl = slice(ns * NQ, (ns + 1) * NQ)
        if split:
            # dc-halves so the first KQ matmuls start after half the bytes
            nc.sync.dma_start(xtb_q[ns][:, 0:4, :], xtb_ext[:, 0:4, qsl])
            nc.sync.dma_start(xtb_q[ns][:, 4:8, :], xtb_ext[:, 4:8, qsl])
        else:
            nc.sync.dma_start(xtb_q[ns][:], xtb_ext[:, :, qsl])

    def emit_wpt():
        for hh in range(2):
            nc.sync.dma_start(wpt[:, 4 * hh:4 * hh + 4, :],
                              wpt_ext[:, 4 * hh:4 * hh + 4, :])

    with tc.tile_pool(name="kqv_ps", bufs=2, space="PSUM") as kqvps, \
         tc.tile_pool(name="strip_ps", bufs=2, space="PSUM") as strip_ps, \
         tc.tile_pool(name="acc_ps", bufs=2, space="PSUM") as acc_ps, \
         tc.tile_pool(name="pt_pool", bufs=4) as pt_pool, \
         tc.tile_pool(name="small", bufs=3) as small, \
         tc.tile_pool(name="bc_pool", bufs=3) as bc_pool, \
         tc.tile_pool(name="saTg_pool", bufs=2) as saTg_pool, \
         tc.tile_pool(name="ost_pool", bufs=3) as ost_pool:

        def emit_warm(tiny, bridge, bcols=128):
            # tiny 8-col matmuls age the PE p-state ramp (~7ns each); the
            # 512-col bridge matmuls keep the busy-streak alive across a
            # known PE-idle window so real matmuls behind them charge at
            # full clock. All write a throwaway [1, x] PSUM row.
            wps = kqvps.tile([P, 512], F32, tag="kqv", name="ps_warm")
            for _ in range(tiny):
                nc.tensor.matmul(wps[0:1, 0:8], lhsT=dw[:], rhs=dx[0:1, 0:8],
                                 start=True, stop=True)
            for _ in range(bridge):
                nc.tensor.matmul(wps[0:1, 0:bcols], lhsT=dw[:],
                                 rhs=dx[0:1, 0:bcols], start=True, stop=True)

        def emit_consts_a():
            # small weights on the SP/HWDGE path, requested before the big
            # x quarters so the DMA-engine FIFO serves them first
            nc.gpsimd.memset(dw[:], 1.0)
            nc.gpsimd.memset(dx[:], 1.0)
            nc.sync.dma_start(wk2[:, 0], wk2_ext[:, 0])
            nc.sync.dma_start(bkq[:], bkq_ext[:])

        def emit_consts_a2():
            nc.sync.dma_start(wq2[:, 0], wq2_ext[:, 0])
            nc.sync.dma_start(wk2[:, 1], wk2_ext[:, 1])
            nc.sync.dma_start(wq2[:, 1], wq2_ext[:, 1])

        def emit_consts_b():
            nc.sync.dma_start(wv[:], wv_ext[:])
            nc.sync.dma_start(vbias[:], vbias_ext[:])
            nc.sync.dma_start(mask[:], mask_dram[:])

        def emit_consts_c():
            nc.gpsimd.dma_start(bproj[:], bproj_ext[:])
            # ones column per head (denominator row of the PV matmul)
            nc.gpsimd.memset(
                v[:].rearrange("p m (h c) -> p m h c", c=HD + 1)[:, :, :, HD:HD + 1],
                1.0,
            )

        def emit_kq_unit(ns, pr, which):
            nsl = slice(ns * 512, (ns + 1) * 512)
            ps = kqvps.tile([P, 512], F32, tag="kqv", name="ps_kq")
            w = wk2 if which == 0 else wq2
            for dc in range(DC):
                nc.tensor.matmul(
                    ps[:], lhsT=w[:, pr, dc, :], rhs=xtb_q[ns][:, dc, :],
                    start=(dc == 0), stop=(dc == DC - 1),
                )
            dst = k2 if which == 0 else q2
            nc.vector.tensor_scalar(
                out=dst[:, pr, nsl], in0=ps[:],
                scalar1=bkq[:, pr, which:which + 1], scalar2=None,
                op0=mybir.AluOpType.add,
            )

        def emit_v_unit(ns, mb):
            msl = slice((mb % 4) * P, (mb % 4 + 1) * P)
            ps = kqvps.tile([P, 512], F32, tag="kqv", name="ps_v")
            for dc in range(DC):
                nc.tensor.matmul(
                    ps[:, :HPC * HD], lhsT=xtb_q[ns][:, dc, msl],
                    rhs=wv[:, dc, :],
                    start=(dc == 0), stop=(dc == DC - 1),
                )
            nc.vector.tensor_tensor(
                out=v[:].rearrange("p m (h c) -> p m h c", c=HD + 1)[:, mb, :, 0:HD],
                in0=ps[:, :HPC * HD].rearrange("p (h e) -> p h e", e=HD),
                in1=vbias[:].rearrange("p (h e) -> p h e", e=HD),
                op=mybir.AluOpType.add,
            )

        def kqv_units(ns, mbs=None, kq=True):
            units = []
            if kq:
                for pr in range(2):
                    for which in range(2):
                        units.append(lambda ns=ns, pr=pr, which=which:
                                     emit_kq_unit(ns, pr, which))
            if mbs is None:
                mbs = range(4 * ns, 4 * ns + 4)
            for mb in mbs:
                units.append(lambda ns=ns, mb=mb: emit_v_unit(ns, mb))
            return units

        # filler machinery: KQV work units dripped into the attention stream
        filler_q = []
        fill_stat = {"queued": 0, "popped": 0}
        tick_n = [1]

        def enqueue_fillers(units):
            filler_q.extend(units)
            fill_stat["queued"] += len(units)
            return fill_stat["queued"]

        def round_tick():
            for _ in range(tick_n[0]):
                if filler_q:
                    filler_q.pop(0)()
                    fill_stat["popped"] += 1

        def flush_through(target):
            while fill_stat["popped"] < target and filler_q:
                filler_q.pop(0)()
                fill_stat["popped"] += 1

        def flush_fillers():
            while filler_q:
                filler_q.pop(0)()
                fill_stat["popped"] += 1

        def emit_attention_block(J):
            """Two heads of each partition-pair in lockstep; PV lags the
            S/exp pipeline so ACT overlaps PE."""
            nsl = slice(J * NB, (J + 1) * NB)
            n_mb = 2 * (J + 1)
            for pr in range(2):
                heads = []
                for h2 in range(2):
                    # one PSUM bank per head; den row at partition 64. Late
                    # blocks borrow the idle kqv pool for pr=1 so the second
                    # pair never waits on the first pair's bank release.
                    if J >= 6 and pr == 1:
                        bank = kqvps.tile([P, 512], F32, tag="kqv",
                                          name=f"ps_acc{h2}")
                    else:
                        bank = acc_ps.tile([P, 512], F32, tag="acc",
                                           name=f"ps_acc{h2}")
                    heads.append({"h2": h2, "l": 2 * pr + h2,
                                  "prow": slice(HD * h2, HD * h2 + HD),
                                  "opsf": bank[:, 0:NB]})
                pending = []  # (head, c0, cn, pts)

                def emit_s(hd, c0, cn):
                    # the last chunk holds the diagonal pair; block 2J+1
                    # only computes (and exps) its valid upper n-half
                    has_diag = c0 <= 2 * J < c0 + cn
                    wid = cn * NB - (128 if has_diag else 0)
                    sps = strip_ps.tile(
                        [P, CHUNK * NB], F32, tag="strip", name="ps_strip"
                    )[:, :wid]
                    for a in range(c0, c0 + cn):
                        o = (a - c0) * NB
                        if has_diag and a == 2 * J + 1:
                            nc.tensor.matmul(
                                sps[:, o:o + 128],
                                lhsT=q2[hd["prow"], pr, a * P:(a + 1) * P],
                                rhs=k2[hd["prow"], pr,
                                       J * NB + 128:(J + 1) * NB],
                                start=True, stop=True,
                            )
                        else:
                            nc.tensor.matmul(
                                sps[:, o:o + NB],
                                lhsT=q2[hd["prow"], pr, a * P:(a + 1) * P],
                                rhs=k2[hd["prow"], pr, nsl],
                                start=True, stop=True,
                            )
                    pts = pt_pool.tile(
                        [P, CHUNK * NB], BF16, tag="pt", name="pt"
                    )[:, :wid]
                    nc.scalar.activation(
                        pts, sps, mybir.ActivationFunctionType.Exp,
                        scale=1.0 / np.sqrt(HD),
                    )
                    if has_diag:
                        o = (2 * J - c0) * NB
                        nc.vector.tensor_tensor(
                            out=pts[:, o:o + 384], in0=pts[:, o:o + 384],
                            in1=mask[:], op=mybir.AluOpType.mult,
                        )
                    pending.append((hd, c0, cn, pts, has_diag))

                def emit_pv(hd, c0, cn, pts, has_diag):
                    lcol = hd["l"] * (HD + 1)
                    for a in range(c0, c0 + cn):
                        o = (a - c0) * NB
                        if has_diag and a == 2 * J + 1:
                            nc.tensor.matmul(
                                hd["opsf"][0:HD + 1, 128:NB],
                                lhsT=v[:, a, lcol:lcol + HD + 1],
                                rhs=pts[:, o:o + 128],
                                start=False, stop=(a == n_mb - 1),
                            )
                        else:
                            nc.tensor.matmul(
                                hd["opsf"][0:HD + 1, :],
                                lhsT=v[:, a, lcol:lcol + HD + 1],
                                rhs=pts[:, o:o + NB],
                                start=(a == 0), stop=(a == n_mb - 1),
                            )

                first = True
                for c0 in range(0, n_mb, CHUNK):
                    cn = min(CHUNK, n_mb - c0)
                    if not first:
                        round_tick()
                    emit_s(heads[0], c0, cn)
                    emit_s(heads[1], c0, cn)
                    while len(pending) > 3:
                        emit_pv(*pending.pop(0))
                    first = False
                while pending:
                    emit_pv(*pending.pop(0))

                # finalize: rc = 1/den, gpsimd broadcast to 64 partitions
                # (SBUF), one normalize-mult PSUM x SBUF -> saT bf16.
                for hd in heads:
                    round_tick()
                    h2 = hd["h2"]
                    rc = small.tile([1, NB], F32, tag="rc", name="rc")
                    nc.vector.reciprocal(rc[:], hd["opsf"][HD:HD + 1, :])
                    bc = bc_pool.tile([HD, NB], F32, tag="bc", name="bc")
                    nc.gpsimd.partition_broadcast(bc[:], rc[:], channels=HD)
                    nc.vector.tensor_tensor(
                        out=saT[hd["prow"], pr, nsl],
                        in0=hd["opsf"][0:HD, :],
                        in1=bc[:],
                        op=mybir.AluOpType.mult,
                    )

        def emit_a2a_stage(k):
            # t=0 (pr0, finalizes first) and t=1 on separate engine queues
            # so the two staging DMAs overlap
            nsl = slice(RBASE[k], RBASE[k] + RANGES[k])
            for t, eng in ((0, nc.sync), (1, nc.sync)):
                eng.dma_start(
                    cc_in[k][:, t].rearrange("s p r -> p s r"),
                    saT[:, t, nsl].rearrange("p (s r) -> p s r", r=RSUB[k]),
                )

        def emit_a2a(k):
            # 2-D receiver-major APs: byte-contiguous (HW requirement)
            # with the per-receiver chunk dim explicit
            nc.gpsimd.collective_compute(
                "AllToAll", mybir.AluOpType.bypass,
                replica_groups=GROUP8,
                ins=[cc_in_f[k][:].rearrange("s t p r -> s (t p r)")],
                outs=[cc_out_f[k][:].rearrange("s t p r -> s (t p r)")],
            )

        proj_mms = {}

        def proj_units(k):
            # saTg[p, fc, rcol]: fc = gs*2 + t (feature chunk),
            # rcol = bs*sub + r over both batches = 2*sub columns
            sub = RSUB[k]
            saTg = saTg_pool.tile([P, DC, 2 * max(RSUB)], BF16, tag="saTg",
                                  name="saTg")[:, :, :2 * sub]

            def load_unit():
                for bs in range(2):
                    nc.sync.dma_start(
                        saTg[:, :, bs * sub:(bs + 1) * sub].rearrange(
                            "p (gs t) r -> p gs t r", t=2),
                        cc_out[k][4 * bs:4 * bs + 4, :].rearrange(
                            "gs t p r -> p gs t r"),
                    )

            def mm_unit(s, half):
                rows = min(P, 2 * sub - s * P)
                pps = kqvps.tile([P, 512], F32, tag="kqv", name="ps_proj")
                hsl = slice(half * 512, (half + 1) * 512)
                for f in range(DC):
                    nc.tensor.matmul(
                        pps[:rows],
                        lhsT=saTg[:, f, s * P:s * P + rows],
                        rhs=wpt[:, f, hsl],
                        start=(f == 0), stop=(f == DC - 1),
                    )
                ost = ost_pool.tile([P, 512], F32, tag="ost", name="ost")
                nc.vector.tensor_tensor(
                    out=ost[:rows], in0=pps[:rows],
                    in1=bproj[:rows, hsl], op=mybir.AluOpType.add,
                )
                nc.sync.dma_start(
                    out[OBASE[k] + s * P:OBASE[k] + s * P + rows, hsl],
                    ost[:rows],
                )

            def mm_unit_q(s, qtr):
                # 256-col accumulation group: shorter bias->out tail chain
                pps = kqvps.tile([P, 512], F32, tag="kqv", name="ps_projq")
                rows = min(P, 2 * sub - s * P)
                qsl = slice(qtr * NB, (qtr + 1) * NB)
                for f in range(DC):
                    nc.tensor.matmul(
                        pps[:rows, 0:NB],
                        lhsT=saTg[:, f, s * P:s * P + rows],
                        rhs=wpt[:, f, qsl],
                        start=(f == 0), stop=(f == DC - 1),
                    )
                ost = ost_pool.tile([P, 512], F32, tag="ost",
                                    name="ost")[:, 0:NB]
                nc.vector.tensor_tensor(
                    out=ost[:rows], in0=pps[:rows, 0:NB],
                    in1=bproj[:rows, qsl], op=mybir.AluOpType.add,
                )
                nc.sync.dma_start(
                    out[OBASE[k] + s * P:OBASE[k] + s * P + rows, qsl],
                    ost[:rows],
                )

            units = []
            n_strip = (2 * sub + P - 1) // P
            for s in range(n_strip):
                if k == len(RANGES) - 1:
                    for qtr in range(4):
                        units.append(lambda s=s, qtr=qtr: mm_unit_q(s, qtr))
                else:
                    for half in range(2):
                        units.append(lambda s=s, half=half: mm_unit(s, half))
            return load_unit, units

        def emit_proj_load(k):
            load, mms = proj_units(k)
            load()
            proj_mms[k] = mms

        def proj_filler_units(k):
            return proj_mms.pop(k)

        def emit_proj_mms(k):
            for u in proj_mms.pop(k):
                u()

        # ---------------- emission order ----------------
        emit_consts_a()
        emit_warm(24, 42)
        emit_x_load(0, split=True)
        emit_consts_a2()
        emit_consts_b()
        emit_x_load(1)
        emit_x_load(2)
        emit_x_load(3)
        for u in kqv_units(0):
            u()
        emit_consts_c()
        mark1 = enqueue_fillers(kqv_units(1))
        tick_n[0] = 2
        emit_attention_block(0)
        mark2 = enqueue_fillers(kqv_units(2))
        emit_attention_block(1)
        tick_n[0] = 1
        flush_through(mark1)     # kqv(1) must complete before attn(2)
        emit_attention_block(2)
        mark3 = enqueue_fillers(kqv_units(3, mbs=[12, 13]))
        emit_attention_block(3)
        flush_through(mark2)     # kqv(2) must complete before attn(4)
        emit_wpt()
        emit_attention_block(4)
        emit_attention_block(5)
        flush_through(mark3)     # KQ(3)+V(12,13) must precede attn(6)
        emit_a2a_stage(0)
        emit_a2a(0)              # rows 0:1536 overlap attn(6)+attn(7)
        enqueue_fillers(kqv_units(3, mbs=[14, 15], kq=False))
        emit_attention_block(6)
        flush_fillers()          # V(14,15) must precede attn(7)
        emit_attention_block(7)
        # stage+fire a2a(1) FIRST: any load emitted earlier would hold the
        # SP sequencer through its wait and delay this staging
        emit_a2a_stage(1)
        emit_a2a(1)              # rows 1536:2048
        # bridge the a2a(0) spill window so proj(0) dispatches warm,
        # then proj(0) + a second bridge fill the a2a(1) window
        emit_warm(0, 26, bcols=512)
        emit_proj_load(0)
        emit_proj_mms(0)
        emit_warm(0, 40, bcols=512)
        emit_proj_load(1)
        emit_proj_mms(1)


def build_nc():
    nc = bacc.Bacc(
        "TRN2", target_bir_lowering=False, debug=False,
        num_devices=N_CORES, enable_asserts=False,
    )
    with tile.TileContext(nc) as tc:
        import contextlib
        with contextlib.ExitStack() as ctx:
            build_kernel(tc, ctx)
    nc.finalize()
    return nc


def make_in_maps(x, W_kqv, b_kqv, W_proj, b_proj):
    """Host-side shard + bf16 cast + layout packing (no math beyond rounding)."""
    in_maps = []
    # wpt[p, f, j] = W_proj[j, f*128+p]
    wpt = np.ascontiguousarray(
        np.asarray(W_proj, np.float32).T.reshape(DC, P, D).transpose(1, 0, 2)
    ).astype(BF16_NP)
    bp_rep = np.ascontiguousarray(
        np.broadcast_to(np.asarray(b_proj, np.float32)[None, :], (P, D)))
    for c in range(N_CORES):
        b = c // 4
        g = c % 4
        wl = np.ascontiguousarray(W_kqv[4 * g:4 * g + 4], np.float32)
        bl = np.ascontiguousarray(b_kqv[4 * g:4 * g + 4], np.float32)
        # [p, l, dc, e] view of the per-head weights (l = local head)
        wr = wl.reshape(HPC, DC, P, 3 * HD).transpose(2, 0, 1, 3)
        # wk2/wq2: [p, pr, dc, h2*64+e]
        wk2 = np.ascontiguousarray(
            wr[:, :, :, 0:HD].reshape(P, 2, 2, DC, HD).transpose(0, 1, 3, 2, 4)
            .reshape(P, 2, DC, P)).astype(BF16_NP)
        wq2 = np.ascontiguousarray(
            wr[:, :, :, HD:2 * HD].reshape(P, 2, 2, DC, HD).transpose(0, 1, 3, 2, 4)
            .reshape(P, 2, DC, P)).astype(BF16_NP)
        # wv: [p, dc, l*64+e]
        wv = np.ascontiguousarray(
            wr[:, :, :, 2 * HD:3 * HD].transpose(0, 2, 1, 3)
            .reshape(P, DC, HPC * HD)).astype(BF16_NP)
        # bkq: [64*h2+e, pr, {k,q}]
        bkq = np.zeros((P, 2, 2), np.float32)
        for pr in range(2):
            for h2 in range(2):
                l = 2 * pr + h2
                bkq[64 * h2:64 * h2 + 64, pr, 0] = bl[l, 0:HD]
                bkq[64 * h2:64 * h2 + 64, pr, 1] = bl[l, HD:2 * HD]
        vbias = np.ascontiguousarray(
            np.broadcast_to(bl[:, 2 * HD:3 * HD].reshape(1, HPC * HD),
                            (P, HPC * HD)))
        # xT: [p, dc, n] = x[b, n, dc*128+p]
        xt = np.ascontiguousarray(
            np.asarray(x[b], np.float32).T.reshape(DC, P, N).transpose(1, 0, 2))
        in_maps.append({
            "xtb": xt.astype(BF16_NP),
            "wk2": wk2,
            "wq2": wq2,
            "wv": wv,
            "bkq": bkq,
            "vbias": vbias,
            "bproj": bp_rep,
            "wpt": wpt,
        })
    return in_maps


def assemble(results):
    full = np.zeros((2, N, D), dtype=np.float32)
    for c in range(N_CORES):
        o = results[c]["out"]
        for k in range(len(RANGES)):
            sub = RSUB[k]
            r0 = RBASE[k] + sub * c
            for b in range(2):
                full[b, r0:r0 + sub, :] = \
                    o[OBASE[k] + sub * b:OBASE[k] + sub * (b + 1), :]
    return full


def kernel(x, W_kqv, b_kqv, W_proj, b_proj):
    x = np.asarray(x)
    W_kqv = np.asarray(W_kqv)
    b_kqv = np.asarray(b_kqv)
    W_proj = np.asarray(W_proj)
    b_proj = np.asarray(b_proj)
    nc = build_nc()
    in_maps = make_in_maps(x, W_kqv, b_kqv, W_proj, b_proj)
    res = run_bass_kernel_spmd(nc, in_maps, list(range(N_CORES)))
    return assemble(res.results)


if __name__ == "__main__":
    rng = np.random.default_rng(0)
    x = rng.standard_normal((2, N, D), dtype=np.float32)
    W_kqv = rng.standard_normal((H, D, 3 * HD), dtype=np.float32) / 32
    b_kqv = rng.standard_normal((H, 3 * HD), dtype=np.float32) / 32
    W_proj = rng.standard_normal((D, D), dtype=np.float32) / 32
    b_proj = rng.standard_normal((D,), dtype=np.float32) / 32
    out = kernel(x, W_kqv, b_kqv, W_proj, b_proj)
    print(out.shape, out.dtype, np.abs(out).max())


# revision 50
# speedup vs baseline: 1.0018x; 1.0018x over previous
"""Trainium2 Bass kernel for nn_CausalSelfAttention (B=2, N=2048, D=1024, H=16).

Sharding (8 cores): batch (2-way) x head-group (4-way, 4 heads per core).
Reference swaps K/Q roles: scores = K @ Q^T, softmax over the Q index.

v3 design (all-bf16 compute; fp8 measurably exceeds the 2e-2 gate on the
K/Q path because softmax averaging does not suppress relative weight
jitter):
- Host pre-transposes x and W_proj (no on-device DMA transposes); all x
  quarters stream in up-front while compute proceeds.
- Attention starts right after quarter-0 KQV: strips 0-1 only need the
  first n/m quarter, so the remaining KQV quarters drip into the strip
  pipeline's stall points as fillers (ACT exp and PE matmuls stay
  co-busy from ~4us on).
- ACT runs exp only; K/Q/V bias adds + casts run on DVE straight from
  PSUM (tensor_scalar / tensor_tensor).
- Softmax normalize: DVE reciprocal of the PV ones-row, gpsimd
  partition_broadcast to 64 partitions (SBUF), then one DVE
  tensor_tensor mult PSUM x SBUF -> saT bf16. No PE broadcast matmul.
- AllToAll ranges [1024, 512, 512] fire after strips 3/5/7; the 4-D
  bounce-buffer APs keep the per-shard layout explicit. Projection
  matmuls for ranges 0-1 execute inside the last exchange's window so
  only the 512-row projection trails it.
"""

import sys

import numpy as np

if "/opt/trn_rl_repo" not in sys.path:
    sys.path.insert(0, "/opt/trn_rl_repo")

import ml_dtypes
import concourse.bass as bass
import concourse.mybir as mybir
import concourse.tile as tile
from concourse import bacc
from concourse.bass_utils import run_bass_kernel_spmd

F32 = mybir.dt.float32
BF16 = mybir.dt.bfloat16
BF16_NP = ml_dtypes.bfloat16

P = 128
N = 2048          # sequence length
D = 1024          # model dim
H = 16            # total heads
HPC = 4           # heads per core
HD = 64           # head dim
DC = D // P       # 8 d-chunks
NB = 256          # attention n-block (free dim of S^T tiles)
NBLK = N // NB    # 8 strips
MB = N // P       # 16 m-blocks
CHUNK = 4         # m-blocks per PSUM strip (4*256 fp32 = 2 PSUM banks)
N_CORES = 8

RANGES = [1536, 512]
RBASE = [0, 1536]
RSUB = [r // 8 for r in RANGES]          # rows per core per range
OBASE = [0, 384]                         # output row base per range (per core)

GROUP8 = [[0, 1, 2, 3, 4, 5, 6, 7]]


def _mask_np():
    # causal mask for the diagonal m-block pair of each strip:
    # cols 0:256   (m_blk 2J,   m = 256J + p, n-cols 0:256)   keep j >= p
    # cols 256:384 (m_blk 2J+1, m = 256J+128+p, n-cols 128:256) keep j >= p
    # (the S matmul for block 2J+1 only computes the upper n-half)
    p = np.arange(P)[:, None]
    m0 = (np.arange(256)[None, :] >= p).astype(np.float32)
    m1 = (np.arange(128)[None, :] >= p).astype(np.float32)
    return np.concatenate([m0, m1], axis=1).astype(BF16_NP)


def build_kernel(tc: tile.TileContext, ctx):
    nc = tc.nc

    xtb_ext = nc.dram_tensor("xtb", [P, DC, N], BF16, kind="ExternalInput")
    wk2_ext = nc.dram_tensor("wk2", [P, 2, DC, P], BF16, kind="ExternalInput")
    wq2_ext = nc.dram_tensor("wq2", [P, 2, DC, P], BF16, kind="ExternalInput")
    wv_ext = nc.dram_tensor("wv", [P, DC, HPC * HD], BF16, kind="ExternalInput")
    bkq_ext = nc.dram_tensor("bkq", [P, 2, 2], F32, kind="ExternalInput")
    vbias_ext = nc.dram_tensor("vbias", [P, HPC * HD], F32, kind="ExternalInput")
    bproj_ext = nc.dram_tensor("bproj", [P, D], F32, kind="ExternalInput")
    wpt_ext = nc.dram_tensor("wpt", [P, DC, D], BF16, kind="ExternalInput")
    out_ext = nc.dram_tensor("out", [512, D], F32, kind="ExternalOutput")

    out = out_ext[:]

    mask_dram = nc.inline_tensor(_mask_np(), name="mask_c")

    dram = ctx.enter_context(tc.tile_pool(name="dram", bufs=1, space="DRAM"))
    const = ctx.enter_context(tc.tile_pool(name="const", bufs=1))

    # AllToAll buffers: [8 chunks (receiver), 2 t, 128 p, sub r] bf16.
    # Row-padded so the per-receiver leading dim stays explicit in the
    # lowered access pattern (strided slices don't collapse).
    CPAD = 0
    cc_in_f = [dram.tile([8, 2, P, RSUB[k] + CPAD], BF16, name=f"cc_in{k}")
               for k in range(len(RANGES))]
    cc_out_f = [dram.tile([8, 2, P, RSUB[k] + CPAD], BF16, name=f"cc_out{k}")
                for k in range(len(RANGES))]
    cc_in = [t[:, :, :, 0:RSUB[k]] for k, t in enumerate(cc_in_f)]
    cc_out = [t[:, :, :, 0:RSUB[k]] for k, t in enumerate(cc_out_f)]

    # ---------------- SBUF constants / weights ----------------
    # per-quarter x^T tiles: separate tiles keep the dependency tracker's
    # byte-range bounding boxes quarter-precise (a unified tile serializes
    # the whole pipeline behind the last x quarter's DMA)
    xtb_q = [const.tile([P, DC, N // 4], BF16, name=f"xtb{q}")
             for q in range(4)]
    wk2 = const.tile([P, 2, DC, P], BF16, name="wk2")
    wq2 = const.tile([P, 2, DC, P], BF16, name="wq2")
    wv = const.tile([P, DC, HPC * HD], BF16, name="wv")
    bkq = const.tile([P, 2, 2], F32, name="bkq")
    vbias = const.tile([P, HPC * HD], F32, name="vbias")
    bproj = const.tile([P, D], F32, name="bproj")
    mask = const.tile([P, 384], BF16, name="mask")
    wpt = const.tile([P, DC, D], BF16, name="wpt")

    # PE p-state warmup dummies (memset-initialized, no DMA dependency)
    dw = const.tile([1, 1], BF16, name="dw")
    dx = const.tile([1, 512], BF16, name="dx")

    # ---------------- KQV / attention state ----------------
    # k2/q2: [64*h2+e, pr, n] bf16 (packed head-pair partition layout)
    k2 = const.tile([P, 2, N], BF16, name="k2")
    q2 = const.tile([P, 2, N], BF16, name="q2")
    v = const.tile([P, MB, HPC * (HD + 1)], BF16, name="v")
    saT = const.tile([P, 2, N], BF16, name="saT")

    NQ = N // 4

    def emit_x_load(ns, split=False):
        qsl = slice(ns * NQ, (ns + 1) * NQ)
        if split:
            # dc-halves so the first KQ matmuls start after half the bytes
            nc.sync.dma_start(xtb_q[ns][:, 0:4, :], xtb_ext[:, 0:4, qsl])
            nc.sync.dma_start(xtb_q[ns][:, 4:8, :], xtb_ext[:, 4:8, qsl])
        else:
            nc.sync.dma_start(xtb_q[ns][:], xtb_ext[:, :, qsl])

    def emit_wpt():
        for hh in range(2):
            nc.sync.dma_start(wpt[:, 4 * hh:4 * hh + 4, :],
                              wpt_ext[:, 4 * hh:4 * hh + 4, :])

    with tc.tile_pool(name="kqv_ps", bufs=2, space="PSUM") as kqvps, \
         tc.tile_pool(name="strip_ps", bufs=2, space="PSUM") as strip_ps, \
         tc.tile_pool(name="acc_ps", bufs=2, space="PSUM") as acc_ps, \
         tc.tile_pool(name="pt_pool", bufs=4) as pt_pool, \
         tc.tile_pool(name="small", bufs=3) as small, \
         tc.tile_pool(name="bc_pool", bufs=3) as bc_pool, \
         tc.tile_pool(name="saTg_pool", bufs=2) as saTg_pool, \
         tc.tile_pool(name="ost_pool", bufs=3) as ost_pool:

        def emit_warm(tiny, bridge, bcols=128):
            # tiny 8-col matmuls age the PE p-state ramp (~7ns each); the
            # 512-col bridge matmuls keep the busy-streak alive across a
            # known PE-idle window so real matmuls behind them charge at
            # full clock. All write a throwaway [1, x] PSUM row.
            wps = kqvps.tile([P, 512], F32, tag="kqv", name="ps_warm")
            for _ in range(tiny):
                nc.tensor.matmul(wps[0:1, 0:8], lhsT=dw[:], rhs=dx[0:1, 0:8],
                                 start=True, stop=True)
            for _ in range(bridge):
                nc.tensor.matmul(wps[0:1, 0:bcols], lhsT=dw[:],
                                 rhs=dx[0:1, 0:bcols], start=True, stop=True)

        def emit_consts_a():
            # small weights on the SP/HWDGE path, requested before the big
            # x quarters so the DMA-engine FIFO serves them first
            nc.gpsimd.memset(dw[:], 1.0)
            nc.gpsimd.memset(dx[:], 1.0)
            nc.sync.dma_start(wk2[:, 0], wk2_ext[:, 0])
            nc.sync.dma_start(bkq[:], bkq_ext[:])

        def emit_consts_a2():
            nc.sync.dma_start(wq2[:, 0], wq2_ext[:, 0])
            nc.sync.dma_start(wk2[:, 1], wk2_ext[:, 1])
            nc.sync.dma_start(wq2[:, 1], wq2_ext[:, 1])

        def emit_consts_b():
            nc.sync.dma_start(wv[:], wv_ext[:])
            nc.sync.dma_start(vbias[:], vbias_ext[:])
            nc.sync.dma_start(mask[:], mask_dram[:])

        def emit_consts_c():
            nc.gpsimd.dma_start(bproj[:], bproj_ext[:])
            # ones column per head (denominator row of the PV matmul)
            nc.gpsimd.memset(
                v[:].rearrange("p m (h c) -> p m h c", c=HD + 1)[:, :, :, HD:HD + 1],
                1.0,
            )

        def emit_kq_unit(ns, pr, which):
            nsl = slice(ns * 512, (ns + 1) * 512)
            ps = kqvps.tile([P, 512], F32, tag="kqv", name="ps_kq")
            w = wk2 if which == 0 else wq2
            for dc in range(DC):
                nc.tensor.matmul(
                    ps[:], lhsT=w[:, pr, dc, :], rhs=xtb_q[ns][:, dc, :],
                    start=(dc == 0), stop=(dc == DC - 1),
                )
            dst = k2 if which == 0 else q2
            nc.vector.tensor_scalar(
                out=dst[:, pr, nsl], in0=ps[:],
                scalar1=bkq[:, pr, which:which + 1], scalar2=None,
                op0=mybir.AluOpType.add,
            )

        def emit_v_unit(ns, mb):
            msl = slice((mb % 4) * P, (mb % 4 + 1) * P)
            ps = kqvps.tile([P, 512], F32, tag="kqv", name="ps_v")
            for dc in range(DC):
                nc.tensor.matmul(
                    ps[:, :HPC * HD], lhsT=xtb_q[ns][:, dc, msl],
                    rhs=wv[:, dc, :],
                    start=(dc == 0), stop=(dc == DC - 1),
                )
            nc.vector.tensor_tensor(
                out=v[:].rearrange("p m (h c) -> p m h c", c=HD + 1)[:, mb, :, 0:HD],
                in0=ps[:, :HPC * HD].rearrange("p (h e) -> p h e", e=HD),
                in1=vbias[:].rearrange("p (h e) -> p h e", e=HD),
                op=mybir.AluOpType.add,
            )

        def kqv_units(ns, mbs=None, kq=True):
            units = []
            if kq:
                for pr in range(2):
                    for which in range(2):
                        units.append(lambda ns=ns, pr=pr, which=which:
                                     emit_kq_unit(ns, pr, which))
            if mbs is None:
                mbs = range(4 * ns, 4 * ns + 4)
            for mb in mbs:
                units.append(lambda ns=ns, mb=mb: emit_v_unit(ns, mb))
            return units

        # filler machinery: KQV work units dripped into the attention stream
        filler_q = []
        fill_stat = {"queued": 0, "popped": 0}
        tick_n = [1]

        def enqueue_fillers(units):
            filler_q.extend(units)
            fill_stat["queued"] += len(units)
            return fill_stat["queued"]

        def round_tick():
            for _ in range(tick_n[0]):
                if filler_q:
                    filler_q.pop(0)()
                    fill_stat["popped"] += 1

        def flush_through(target):
            while fill_stat["popped"] < target and filler_q:
                filler_q.pop(0)()
                fill_stat["popped"] += 1

        def flush_fillers():
            while filler_q:
                filler_q.pop(0)()
                fill_stat["popped"] += 1

        def emit_attention_block(J):
            """Two heads of each partition-pair in lockstep; PV lags the
            S/exp pipeline so ACT overlaps PE."""
            nsl = slice(J * NB, (J + 1) * NB)
            n_mb = 2 * (J + 1)
            for pr in range(2):
                heads = []
                for h2 in range(2):
                    # one PSUM bank per head; den row at partition 64. Late
                    # blocks borrow the idle kqv pool for pr=1 so the second
                    # pair never waits on the first pair's bank release.
                    if J >= 6 and pr == 1:
                        bank = kqvps.tile([P, 512], F32, tag="kqv",
                                          name=f"ps_acc{h2}")
                    else:
                        bank = acc_ps.tile([P, 512], F32, tag="acc",
                                           name=f"ps_acc{h2}")
                    heads.append({"h2": h2, "l": 2 * pr + h2,
                                  "prow": slice(HD * h2, HD * h2 + HD),
                                  "opsf": bank[:, 0:NB]})
                pending = []  # (head, c0, cn, pts)

                def emit_s(hd, c0, cn):
                    # the last chunk holds the diagonal pair; block 2J+1
                    # only computes (and exps) its valid upper n-half
                    has_diag = c0 <= 2 * J < c0 + cn
                    wid = cn * NB - (128 if has_diag else 0)
                    sps = strip_ps.tile(
                        [P, CHUNK * NB], F32, tag="strip", name="ps_strip"
                    )[:, :wid]
                    for a in range(c0, c0 + cn):
                        o = (a - c0) * NB
                        if has_diag and a == 2 * J + 1:
                            nc.tensor.matmul(
                                sps[:, o:o + 128],
                                lhsT=q2[hd["prow"], pr, a * P:(a + 1) * P],
                                rhs=k2[hd["prow"], pr,
                                       J * NB + 128:(J + 1) * NB],
                                start=True, stop=True,
                            )
                        else:
                            nc.tensor.matmul(
                                sps[:, o:o + NB],
                                lhsT=q2[hd["prow"], pr, a * P:(a + 1) * P],
                                rhs=k2[hd["prow"], pr, nsl],
                                start=True, stop=True,
                            )
                    pts = pt_pool.tile(
                        [P, CHUNK * NB], BF16, tag="pt", name="pt"
                    )[:, :wid]
                    nc.scalar.activation(
                        pts, sps, mybir.ActivationFunctionType.Exp,
                        scale=1.0 / np.sqrt(HD),
                    )
                    if has_diag:
                        o = (2 * J - c0) * NB
                        nc.vector.tensor_tensor(
                            out=pts[:, o:o + 384], in0=pts[:, o:o + 384],
                            in1=mask[:], op=mybir.AluOpType.mult,
                        )
                    pending.append((hd, c0, cn, pts, has_diag))

                def emit_pv(hd, c0, cn, pts, has_diag):
                    lcol = hd["l"] * (HD + 1)
                    for a in range(c0, c0 + cn):
                        o = (a - c0) * NB
                        if has_diag and a == 2 * J + 1:
                            nc.tensor.matmul(
                                hd["opsf"][0:HD + 1, 128:NB],
                                lhsT=v[:, a, lcol:lcol + HD + 1],
                                rhs=pts[:, o:o + 128],
                                start=False, stop=(a == n_mb - 1),
                            )
                        else:
                            nc.tensor.matmul(
                                hd["opsf"][0:HD + 1, :],
                                lhsT=v[:, a, lcol:lcol + HD + 1],
                                rhs=pts[:, o:o + NB],
                                start=(a == 0), stop=(a == n_mb - 1),
                            )

                first = True
                for c0 in range(0, n_mb, CHUNK):
                    cn = min(CHUNK, n_mb - c0)
                    if not first:
                        round_tick()
                    emit_s(heads[0], c0, cn)
                    emit_s(heads[1], c0, cn)
                    while len(pending) > 3:
                        emit_pv(*pending.pop(0))
                    first = False
                while pending:
                    emit_pv(*pending.pop(0))

                # finalize: rc = 1/den, gpsimd broadcast to 64 partitions
                # (SBUF), one normalize-mult PSUM x SBUF -> saT bf16.
                for hd in heads:
                    round_tick()
                    h2 = hd["h2"]
                    rc = small.tile([1, NB], F32, tag="rc", name="rc")
                    nc.vector.reciprocal(rc[:], hd["opsf"][HD:HD + 1, :])
                    bc = bc_pool.tile([HD, NB], F32, tag="bc", name="bc")
                    nc.gpsimd.partition_broadcast(bc[:], rc[:], channels=HD)
                    nc.vector.tensor_tensor(
                        out=saT[hd["prow"], pr, nsl],
                        in0=hd["opsf"][0:HD, :],
                        in1=bc[:],
                        op=mybir.AluOpType.mult,
                    )

        def emit_a2a_stage(k):
            # t=0 (pr0, finalizes first) and t=1 on separate engine queues
            # so the two staging DMAs overlap
            nsl = slice(RBASE[k], RBASE[k] + RANGES[k])
            for t, eng in ((0, nc.sync), (1, nc.sync)):
                eng.dma_start(
                    cc_in[k][:, t].rearrange("s p r -> p s r"),
                    saT[:, t, nsl].rearrange("p (s r) -> p s r", r=RSUB[k]),
                )

        def emit_a2a(k):
            # 2-D receiver-major APs: byte-contiguous (HW requirement)
            # with the per-receiver chunk dim explicit
            nc.gpsimd.collective_compute(
                "AllToAll", mybir.AluOpType.bypass,
                replica_groups=GROUP8,
                ins=[cc_in_f[k][:].rearrange("s t p r -> s (t p r)")],
                outs=[cc_out_f[k][:].rearrange("s t p r -> s (t p r)")],
            )

        proj_mms = {}

        def proj_units(k):
            # saTg[p, fc, rcol]: fc = gs*2 + t (feature chunk),
            # rcol = bs*sub + r over both batches = 2*sub columns
            sub = RSUB[k]
            saTg = saTg_pool.tile([P, DC, 2 * max(RSUB)], BF16, tag="saTg",
                                  name="saTg")[:, :, :2 * sub]

            def load_unit():
                for bs in range(2):
                    nc.sync.dma_start(
                        saTg[:, :, bs * sub:(bs + 1) * sub].rearrange(
                            "p (gs t) r -> p gs t r", t=2),
                        cc_out[k][4 * bs:4 * bs + 4, :].rearrange(
                            "gs t p r -> p gs t r"),
                    )

            def mm_unit(s, half):
                rows = min(P, 2 * sub - s * P)
                pps = kqvps.tile([P, 512], F32, tag="kqv", name="ps_proj")
                hsl = slice(half * 512, (half + 1) * 512)
                for f in range(DC):
                    nc.tensor.matmul(
                        pps[:rows],
                        lhsT=saTg[:, f, s * P:s * P + rows],
                        rhs=wpt[:, f, hsl],
                        start=(f == 0), stop=(f == DC - 1),
                    )
                ost = ost_pool.tile([P, 512], F32, tag="ost", name="ost")
                nc.vector.tensor_tensor(
                    out=ost[:rows], in0=pps[:rows],
                    in1=bproj[:rows, hsl], op=mybir.AluOpType.add,
                )
                nc.sync.dma_start(
                    out[OBASE[k] + s * P:OBASE[k] + s * P + rows, hsl],
                    ost[:rows],
                )

            def mm_unit_q(s, qtr):
                # 256-col accumulation group: shorter bias->out tail chain
                pps = kqvps.tile([P, 512], F32, tag="kqv", name="ps_projq")
                rows = min(P, 2 * sub - s * P)
                qsl = slice(qtr * NB, (qtr + 1) * NB)
                for f in range(DC):
                    nc.tensor.matmul(
                        pps[:rows, 0:NB],
                        lhsT=saTg[:, f, s * P:s * P + rows],
                        rhs=wpt[:, f, qsl],
                        start=(f == 0), stop=(f == DC - 1),
                    )
                ost = ost_pool.tile([P, 512], F32, tag="ost",
                                    name="ost")[:, 0:NB]
                nc.vector.tensor_tensor(
                    out=ost[:rows], in0=pps[:rows, 0:NB],
                    in1=bproj[:rows, qsl], op=mybir.AluOpType.add,
                )
                nc.sync.dma_start(
                    out[OBASE[k] + s * P:OBASE[k] + s * P + rows, qsl],
                    ost[:rows],
                )

            units = []
            n_strip = (2 * sub + P - 1) // P
            for s in range(n_strip):
                if k == len(RANGES) - 1:
                    for qtr in range(4):
                        units.append(lambda s=s, qtr=qtr: mm_unit_q(s, qtr))
                else:
                    for half in range(2):
                        units.append(lambda s=s, half=half: mm_unit(s, half))
            return load_unit, units

        def emit_proj_load(k):
            load, mms = proj_units(k)
            load()
            proj_mms[k] = mms

        def proj_filler_units(k):
            return proj_mms.pop(k)

        def emit_proj_mms(k):
            for u in proj_mms.pop(k):
                u()

        # ---------------- emission order ----------------
        emit_consts_a()
        emit_warm(24, 36)
        emit_x_load(0, split=True)
        emit_consts_a2()
        emit_consts_b()
        emit_x_load(1)
        emit_x_load(2)
        emit_x_load(3)
        for u in kqv_units(0):
            u()
        emit_consts_c()
        mark1 = enqueue_fillers(kqv_units(1))
        tick_n[0] = 2
        emit_attention_block(0)
        mark2 = enqueue_fillers(kqv_units(2))
        emit_attention_block(1)
        tick_n[0] = 1
        flush_through(mark1)     # kqv(1) must complete before attn(2)
        emit_attention_block(2)
        mark3 = enqueue_fillers(kqv_units(3, mbs=[12, 13]))
        emit_attention_block(3)
        flush_through(mark2)     # kqv(2) must complete before attn(4)
        emit_wpt()
        emit_attention_block(4)
        emit_attention_block(5)
        flush_through(mark3)     # KQ(3)+V(12,13) must precede attn(6)
        emit_a2a_stage(0)
        emit_a2a(0)              # rows 0:1536 overlap attn(6)+attn(7)
        enqueue_fillers(kqv_units(3, mbs=[14, 15], kq=False))
        emit_attention_block(6)
        flush_fillers()          # V(14,15) must precede attn(7)
        emit_attention_block(7)
        # stage+fire a2a(1) FIRST: any load emitted earlier would hold the
        # SP sequencer through its wait and delay this staging
        emit_a2a_stage(1)
        emit_a2a(1)              # rows 1536:2048
        # bridge the a2a(0) spill window so proj(0) dispatches warm,
        # then proj(0) + a second bridge fill the a2a(1) window
        emit_warm(0, 26, bcols=512)
        emit_proj_load(0)
        emit_proj_mms(0)
        emit_warm(0, 40, bcols=512)
        emit_proj_load(1)
        emit_proj_mms(1)


def build_nc():
    nc = bacc.Bacc(
        "TRN2", target_bir_lowering=False, debug=False,
        num_devices=N_CORES, enable_asserts=False,
    )
    with tile.TileContext(nc) as tc:
        import contextlib
        with contextlib.ExitStack() as ctx:
            build_kernel(tc, ctx)
    nc.finalize()
    return nc


def make_in_maps(x, W_kqv, b_kqv, W_proj, b_proj):
    """Host-side shard + bf16 cast + layout packing (no math beyond rounding)."""
    in_maps = []
    # wpt[p, f, j] = W_proj[j, f*128+p]
    wpt = np.ascontiguousarray(
        np.asarray(W_proj, np.float32).T.reshape(DC, P, D).transpose(1, 0, 2)
    ).astype(BF16_NP)
    bp_rep = np.ascontiguousarray(
        np.broadcast_to(np.asarray(b_proj, np.float32)[None, :], (P, D)))
    for c in range(N_CORES):
        b = c // 4
        g = c % 4
        wl = np.ascontiguousarray(W_kqv[4 * g:4 * g + 4], np.float32)
        bl = np.ascontiguousarray(b_kqv[4 * g:4 * g + 4], np.float32)
        # [p, l, dc, e] view of the per-head weights (l = local head)
        wr = wl.reshape(HPC, DC, P, 3 * HD).transpose(2, 0, 1, 3)
        # wk2/wq2: [p, pr, dc, h2*64+e]
        wk2 = np.ascontiguousarray(
            wr[:, :, :, 0:HD].reshape(P, 2, 2, DC, HD).transpose(0, 1, 3, 2, 4)
            .reshape(P, 2, DC, P)).astype(BF16_NP)
        wq2 = np.ascontiguousarray(
            wr[:, :, :, HD:2 * HD].reshape(P, 2, 2, DC, HD).transpose(0, 1, 3, 2, 4)
            .reshape(P, 2, DC, P)).astype(BF16_NP)
        # wv: [p, dc, l*64+e]
        wv = np.ascontiguousarray(
            wr[:, :, :, 2 * HD:3 * HD].transpose(0, 2, 1, 3)
            .reshape(P, DC, HPC * HD)).astype(BF16_NP)
        # bkq: [64*h2+e, pr, {k,q}]
        bkq = np.zeros((P, 2, 2), np.float32)
        for pr in range(2):
            for h2 in range(2):
                l = 2 * pr + h2
                bkq[64 * h2:64 * h2 + 64, pr, 0] = bl[l, 0:HD]
                bkq[64 * h2:64 * h2 + 64, pr, 1] = bl[l, HD:2 * HD]
        vbias = np.ascontiguousarray(
            np.broadcast_to(bl[:, 2 * HD:3 * HD].reshape(1, HPC * HD),
                            (P, HPC * HD)))
        # xT: [p, dc, n] = x[b, n, dc*128+p]
        xt = np.ascontiguousarray(
            np.asarray(x[b], np.float32).T.reshape(DC, P, N).transpose(1, 0, 2))
        in_maps.append({
            "xtb": xt.astype(BF16_NP),
            "wk2": wk2,
            "wq2": wq2,
            "wv": wv,
            "bkq": bkq,
            "vbias": vbias,
            "bproj": bp_rep,
            "wpt": wpt,
        })
    return in_maps


def assemble(results):
    full = np.zeros((2, N, D), dtype=np.float32)
    for c in range(N_CORES):
        o = results[c]["out"]
        for k in range(len(RANGES)):
            sub = RSUB[k]
            r0 = RBASE[k] + sub * c
            for b in range(2):
                full[b, r0:r0 + sub, :] = \
                    o[OBASE[k] + sub * b:OBASE[k] + sub * (b + 1), :]
    return full


def kernel(x, W_kqv, b_kqv, W_proj, b_proj):
    x = np.asarray(x)
    W_kqv = np.asarray(W_kqv)
    b_kqv = np.asarray(b_kqv)
    W_proj = np.asarray(W_proj)
    b_proj = np.asarray(b_proj)
    nc = build_nc()
    in_maps = make_in_maps(x, W_kqv, b_kqv, W_proj, b_proj)
    res = run_bass_kernel_spmd(nc, in_maps, list(range(N_CORES)))
    return assemble(res.results)


if __name__ == "__main__":
    rng = np.random.default_rng(0)
    x = rng.standard_normal((2, N, D), dtype=np.float32)
    W_kqv = rng.standard_normal((H, D, 3 * HD), dtype=np.float32) / 32
    b_kqv = rng.standard_normal((H, 3 * HD), dtype=np.float32) / 32
    W_proj = rng.standard_normal((D, D), dtype=np.float32) / 32
    b_proj = rng.standard_normal((D,), dtype=np.float32) / 32
    out = kernel(x, W_kqv, b_kqv, W_proj, b_proj)
    print(out.shape, out.dtype, np.abs(out).max())


# revision 51
# speedup vs baseline: 1.0060x; 1.0042x over previous
"""Trainium2 Bass kernel for nn_CausalSelfAttention (B=2, N=2048, D=1024, H=16).

Sharding (8 cores): batch (2-way) x head-group (4-way, 4 heads per core).
Reference swaps K/Q roles: scores = K @ Q^T, softmax over the Q index.

v3 design (all-bf16 compute; fp8 measurably exceeds the 2e-2 gate on the
K/Q path because softmax averaging does not suppress relative weight
jitter):
- Host pre-transposes x and W_proj (no on-device DMA transposes); all x
  quarters stream in up-front while compute proceeds.
- Attention starts right after quarter-0 KQV: strips 0-1 only need the
  first n/m quarter, so the remaining KQV quarters drip into the strip
  pipeline's stall points as fillers (ACT exp and PE matmuls stay
  co-busy from ~4us on).
- ACT runs exp only; K/Q/V bias adds + casts run on DVE straight from
  PSUM (tensor_scalar / tensor_tensor).
- Softmax normalize: DVE reciprocal of the PV ones-row, gpsimd
  partition_broadcast to 64 partitions (SBUF), then one DVE
  tensor_tensor mult PSUM x SBUF -> saT bf16. No PE broadcast matmul.
- AllToAll ranges [1024, 512, 512] fire after strips 3/5/7; the 4-D
  bounce-buffer APs keep the per-shard layout explicit. Projection
  matmuls for ranges 0-1 execute inside the last exchange's window so
  only the 512-row projection trails it.
"""

import sys

import numpy as np

if "/opt/trn_rl_repo" not in sys.path:
    sys.path.insert(0, "/opt/trn_rl_repo")

import ml_dtypes
import concourse.bass as bass
import concourse.mybir as mybir
import concourse.tile as tile
from concourse import bacc
from concourse.bass_utils import run_bass_kernel_spmd

F32 = mybir.dt.float32
BF16 = mybir.dt.bfloat16
BF16_NP = ml_dtypes.bfloat16

P = 128
N = 2048          # sequence length
D = 1024          # model dim
H = 16            # total heads
HPC = 4           # heads per core
HD = 64           # head dim
DC = D // P       # 8 d-chunks
NB = 256          # attention n-block (free dim of S^T tiles)
NBLK = N // NB    # 8 strips
MB = N // P       # 16 m-blocks
CHUNK = 4         # m-blocks per PSUM strip (4*256 fp32 = 2 PSUM banks)
N_CORES = 8

RANGES = [1536, 512]
RBASE = [0, 1536]
RSUB = [r // 8 for r in RANGES]          # rows per core per range
OBASE = [0, 384]                         # output row base per range (per core)

GROUP8 = [[0, 1, 2, 3, 4, 5, 6, 7]]


def _mask_np():
    # causal mask for the diagonal m-block pair of each strip:
    # cols 0:256   (m_blk 2J,   m = 256J + p, n-cols 0:256)   keep j >= p
    # cols 256:384 (m_blk 2J+1, m = 256J+128+p, n-cols 128:256) keep j >= p
    # (the S matmul for block 2J+1 only computes the upper n-half)
    p = np.arange(P)[:, None]
    m0 = (np.arange(256)[None, :] >= p).astype(np.float32)
    m1 = (np.arange(128)[None, :] >= p).astype(np.float32)
    return np.concatenate([m0, m1], axis=1).astype(BF16_NP)


def build_kernel(tc: tile.TileContext, ctx):
    nc = tc.nc

    xtb_ext = nc.dram_tensor("xtb", [P, DC, N], BF16, kind="ExternalInput")
    wk2_ext = nc.dram_tensor("wk2", [P, 2, DC, P], BF16, kind="ExternalInput")
    wq2_ext = nc.dram_tensor("wq2", [P, 2, DC, P], BF16, kind="ExternalInput")
    wv_ext = nc.dram_tensor("wv", [P, DC, HPC * HD], BF16, kind="ExternalInput")
    bkq_ext = nc.dram_tensor("bkq", [P, 2, 2], F32, kind="ExternalInput")
    vbias_ext = nc.dram_tensor("vbias", [P, HPC * HD], F32, kind="ExternalInput")
    bproj_ext = nc.dram_tensor("bproj", [P, D], F32, kind="ExternalInput")
    wpt_ext = nc.dram_tensor("wpt", [P, DC, D], BF16, kind="ExternalInput")
    out_ext = nc.dram_tensor("out", [512, D], F32, kind="ExternalOutput")

    out = out_ext[:]

    mask_dram = nc.inline_tensor(_mask_np(), name="mask_c")

    dram = ctx.enter_context(tc.tile_pool(name="dram", bufs=1, space="DRAM"))
    const = ctx.enter_context(tc.tile_pool(name="const", bufs=1))

    # AllToAll buffers: [8 chunks (receiver), 2 t, 128 p, sub r] bf16.
    # Row-padded so the per-receiver leading dim stays explicit in the
    # lowered access pattern (strided slices don't collapse).
    CPAD = 0
    cc_in_f = [dram.tile([8, 2, P, RSUB[k] + CPAD], BF16, name=f"cc_in{k}")
               for k in range(len(RANGES))]
    cc_out_f = [dram.tile([8, 2, P, RSUB[k] + CPAD], BF16, name=f"cc_out{k}")
                for k in range(len(RANGES))]
    cc_in = [t[:, :, :, 0:RSUB[k]] for k, t in enumerate(cc_in_f)]
    cc_out = [t[:, :, :, 0:RSUB[k]] for k, t in enumerate(cc_out_f)]

    # ---------------- SBUF constants / weights ----------------
    # per-quarter x^T tiles: separate tiles keep the dependency tracker's
    # byte-range bounding boxes quarter-precise (a unified tile serializes
    # the whole pipeline behind the last x quarter's DMA)
    xtb_q = [const.tile([P, DC, N // 4], BF16, name=f"xtb{q}")
             for q in range(4)]
    wk2 = const.tile([P, 2, DC, P], BF16, name="wk2")
    wq2 = const.tile([P, 2, DC, P], BF16, name="wq2")
    wv = const.tile([P, DC, HPC * HD], BF16, name="wv")
    bkq = const.tile([P, 2, 2], F32, name="bkq")
    vbias = const.tile([P, HPC * HD], F32, name="vbias")
    bproj = const.tile([P, D], F32, name="bproj")
    mask = const.tile([P, 384], BF16, name="mask")
    wpt = const.tile([P, DC, D], BF16, name="wpt")

    # PE p-state warmup dummies (memset-initialized, no DMA dependency)
    dw = const.tile([1, 1], BF16, name="dw")
    dx = const.tile([1, 512], BF16, name="dx")

    # ---------------- KQV / attention state ----------------
    # k2/q2: [64*h2+e, pr, n] bf16 (packed head-pair partition layout)
    k2 = const.tile([P, 2, N], BF16, name="k2")
    q2 = const.tile([P, 2, N], BF16, name="q2")
    v = const.tile([P, MB, HPC * (HD + 1)], BF16, name="v")
    saT = const.tile([P, 2, N], BF16, name="saT")

    NQ = N // 4

    def emit_x_load(ns, split=False):
        qsl = slice(ns * NQ, (ns + 1) * NQ)
        if split:
            # dc-halves so the first KQ matmuls start after half the bytes
            nc.sync.dma_start(xtb_q[ns][:, 0:4, :], xtb_ext[:, 0:4, qsl])
            nc.sync.dma_start(xtb_q[ns][:, 4:8, :], xtb_ext[:, 4:8, qsl])
        else:
            nc.sync.dma_start(xtb_q[ns][:], xtb_ext[:, :, qsl])

    def emit_wpt():
        for hh in range(2):
            nc.sync.dma_start(wpt[:, 4 * hh:4 * hh + 4, :],
                              wpt_ext[:, 4 * hh:4 * hh + 4, :])

    with tc.tile_pool(name="kqv_ps", bufs=2, space="PSUM") as kqvps, \
         tc.tile_pool(name="strip_ps", bufs=2, space="PSUM") as strip_ps, \
         tc.tile_pool(name="acc_ps", bufs=2, space="PSUM") as acc_ps, \
         tc.tile_pool(name="pt_pool", bufs=6) as pt_pool, \
         tc.tile_pool(name="small", bufs=3) as small, \
         tc.tile_pool(name="bc_pool", bufs=3) as bc_pool, \
         tc.tile_pool(name="saTg_pool", bufs=2) as saTg_pool, \
         tc.tile_pool(name="ost_pool", bufs=3) as ost_pool:

        def emit_warm(tiny, bridge, bcols=128):
            # tiny 8-col matmuls age the PE p-state ramp (~7ns each); the
            # 512-col bridge matmuls keep the busy-streak alive across a
            # known PE-idle window so real matmuls behind them charge at
            # full clock. All write a throwaway [1, x] PSUM row.
            wps = kqvps.tile([P, 512], F32, tag="kqv", name="ps_warm")
            for _ in range(tiny):
                nc.tensor.matmul(wps[0:1, 0:8], lhsT=dw[:], rhs=dx[0:1, 0:8],
                                 start=True, stop=True)
            for _ in range(bridge):
                nc.tensor.matmul(wps[0:1, 0:bcols], lhsT=dw[:],
                                 rhs=dx[0:1, 0:bcols], start=True, stop=True)

        def emit_consts_a():
            # small weights on the SP/HWDGE path, requested before the big
            # x quarters so the DMA-engine FIFO serves them first
            nc.gpsimd.memset(dw[:], 1.0)
            nc.gpsimd.memset(dx[:], 1.0)
            nc.sync.dma_start(wk2[:, 0], wk2_ext[:, 0])
            nc.sync.dma_start(bkq[:], bkq_ext[:])

        def emit_consts_a2():
            nc.sync.dma_start(wq2[:, 0], wq2_ext[:, 0])
            nc.sync.dma_start(wk2[:, 1], wk2_ext[:, 1])
            nc.sync.dma_start(wq2[:, 1], wq2_ext[:, 1])

        def emit_consts_b():
            nc.sync.dma_start(wv[:], wv_ext[:])
            nc.sync.dma_start(vbias[:], vbias_ext[:])
            nc.sync.dma_start(mask[:], mask_dram[:])

        def emit_consts_c():
            nc.gpsimd.dma_start(bproj[:], bproj_ext[:])
            # ones column per head (denominator row of the PV matmul)
            nc.gpsimd.memset(
                v[:].rearrange("p m (h c) -> p m h c", c=HD + 1)[:, :, :, HD:HD + 1],
                1.0,
            )

        def emit_kq_unit(ns, pr, which):
            nsl = slice(ns * 512, (ns + 1) * 512)
            ps = kqvps.tile([P, 512], F32, tag="kqv", name="ps_kq")
            w = wk2 if which == 0 else wq2
            for dc in range(DC):
                nc.tensor.matmul(
                    ps[:], lhsT=w[:, pr, dc, :], rhs=xtb_q[ns][:, dc, :],
                    start=(dc == 0), stop=(dc == DC - 1),
                )
            dst = k2 if which == 0 else q2
            nc.vector.tensor_scalar(
                out=dst[:, pr, nsl], in0=ps[:],
                scalar1=bkq[:, pr, which:which + 1], scalar2=None,
                op0=mybir.AluOpType.add,
            )

        def emit_v_unit(ns, mb):
            msl = slice((mb % 4) * P, (mb % 4 + 1) * P)
            ps = kqvps.tile([P, 512], F32, tag="kqv", name="ps_v")
            for dc in range(DC):
                nc.tensor.matmul(
                    ps[:, :HPC * HD], lhsT=xtb_q[ns][:, dc, msl],
                    rhs=wv[:, dc, :],
                    start=(dc == 0), stop=(dc == DC - 1),
                )
            nc.vector.tensor_tensor(
                out=v[:].rearrange("p m (h c) -> p m h c", c=HD + 1)[:, mb, :, 0:HD],
                in0=ps[:, :HPC * HD].rearrange("p (h e) -> p h e", e=HD),
                in1=vbias[:].rearrange("p (h e) -> p h e", e=HD),
                op=mybir.AluOpType.add,
            )

        def kqv_units(ns, mbs=None, kq=True):
            units = []
            if kq:
                for pr in range(2):
                    for which in range(2):
                        units.append(lambda ns=ns, pr=pr, which=which:
                                     emit_kq_unit(ns, pr, which))
            if mbs is None:
                mbs = range(4 * ns, 4 * ns + 4)
            for mb in mbs:
                units.append(lambda ns=ns, mb=mb: emit_v_unit(ns, mb))
            return units

        # filler machinery: KQV work units dripped into the attention stream
        filler_q = []
        fill_stat = {"queued": 0, "popped": 0}
        tick_n = [1]

        def enqueue_fillers(units):
            filler_q.extend(units)
            fill_stat["queued"] += len(units)
            return fill_stat["queued"]

        def round_tick():
            for _ in range(tick_n[0]):
                if filler_q:
                    filler_q.pop(0)()
                    fill_stat["popped"] += 1

        def flush_through(target):
            while fill_stat["popped"] < target and filler_q:
                filler_q.pop(0)()
                fill_stat["popped"] += 1

        def flush_fillers():
            while filler_q:
                filler_q.pop(0)()
                fill_stat["popped"] += 1

        def emit_attention_block(J):
            """Two heads of each partition-pair in lockstep; PV lags the
            S/exp pipeline so ACT overlaps PE."""
            nsl = slice(J * NB, (J + 1) * NB)
            n_mb = 2 * (J + 1)
            for pr in range(2):
                heads = []
                for h2 in range(2):
                    # one PSUM bank per head; den row at partition 64. Late
                    # blocks borrow the idle kqv pool for pr=1 so the second
                    # pair never waits on the first pair's bank release.
                    if J >= 6 and pr == 1:
                        bank = kqvps.tile([P, 512], F32, tag="kqv",
                                          name=f"ps_acc{h2}")
                    else:
                        bank = acc_ps.tile([P, 512], F32, tag="acc",
                                           name=f"ps_acc{h2}")
                    heads.append({"h2": h2, "l": 2 * pr + h2,
                                  "prow": slice(HD * h2, HD * h2 + HD),
                                  "opsf": bank[:, 0:NB]})
                pending = []  # (head, c0, cn, pts)

                def emit_s(hd, c0, cn):
                    # the last chunk holds the diagonal pair; block 2J+1
                    # only computes (and exps) its valid upper n-half
                    has_diag = c0 <= 2 * J < c0 + cn
                    wid = cn * NB - (128 if has_diag else 0)
                    sps = strip_ps.tile(
                        [P, CHUNK * NB], F32, tag="strip", name="ps_strip"
                    )[:, :wid]
                    for a in range(c0, c0 + cn):
                        o = (a - c0) * NB
                        if has_diag and a == 2 * J + 1:
                            nc.tensor.matmul(
                                sps[:, o:o + 128],
                                lhsT=q2[hd["prow"], pr, a * P:(a + 1) * P],
                                rhs=k2[hd["prow"], pr,
                                       J * NB + 128:(J + 1) * NB],
                                start=True, stop=True,
                            )
                        else:
                            nc.tensor.matmul(
                                sps[:, o:o + NB],
                                lhsT=q2[hd["prow"], pr, a * P:(a + 1) * P],
                                rhs=k2[hd["prow"], pr, nsl],
                                start=True, stop=True,
                            )
                    pts = pt_pool.tile(
                        [P, CHUNK * NB], BF16, tag="pt", name="pt"
                    )[:, :wid]
                    nc.scalar.activation(
                        pts, sps, mybir.ActivationFunctionType.Exp,
                        scale=1.0 / np.sqrt(HD),
                    )
                    if has_diag:
                        o = (2 * J - c0) * NB
                        nc.vector.tensor_tensor(
                            out=pts[:, o:o + 384], in0=pts[:, o:o + 384],
                            in1=mask[:], op=mybir.AluOpType.mult,
                        )
                    pending.append((hd, c0, cn, pts, has_diag))

                def emit_pv(hd, c0, cn, pts, has_diag):
                    lcol = hd["l"] * (HD + 1)
                    for a in range(c0, c0 + cn):
                        o = (a - c0) * NB
                        if has_diag and a == 2 * J + 1:
                            nc.tensor.matmul(
                                hd["opsf"][0:HD + 1, 128:NB],
                                lhsT=v[:, a, lcol:lcol + HD + 1],
                                rhs=pts[:, o:o + 128],
                                start=False, stop=(a == n_mb - 1),
                            )
                        else:
                            nc.tensor.matmul(
                                hd["opsf"][0:HD + 1, :],
                                lhsT=v[:, a, lcol:lcol + HD + 1],
                                rhs=pts[:, o:o + NB],
                                start=(a == 0), stop=(a == n_mb - 1),
                            )

                first = True
                for c0 in range(0, n_mb, CHUNK):
                    cn = min(CHUNK, n_mb - c0)
                    if not first:
                        round_tick()
                    emit_s(heads[0], c0, cn)
                    emit_s(heads[1], c0, cn)
                    while len(pending) > 4:
                        emit_pv(*pending.pop(0))
                    first = False
                while pending:
                    emit_pv(*pending.pop(0))

                # finalize: rc = 1/den, gpsimd broadcast to 64 partitions
                # (SBUF), one normalize-mult PSUM x SBUF -> saT bf16.
                for hd in heads:
                    round_tick()
                    h2 = hd["h2"]
                    rc = small.tile([1, NB], F32, tag="rc", name="rc")
                    nc.vector.reciprocal(rc[:], hd["opsf"][HD:HD + 1, :])
                    bc = bc_pool.tile([HD, NB], F32, tag="bc", name="bc")
                    nc.gpsimd.partition_broadcast(bc[:], rc[:], channels=HD)
                    nc.vector.tensor_tensor(
                        out=saT[hd["prow"], pr, nsl],
                        in0=hd["opsf"][0:HD, :],
                        in1=bc[:],
                        op=mybir.AluOpType.mult,
                    )

        def emit_a2a_stage(k):
            # t=0 (pr0, finalizes first) and t=1 on separate engine queues
            # so the two staging DMAs overlap
            nsl = slice(RBASE[k], RBASE[k] + RANGES[k])
            for t, eng in ((0, nc.sync), (1, nc.sync)):
                eng.dma_start(
                    cc_in[k][:, t].rearrange("s p r -> p s r"),
                    saT[:, t, nsl].rearrange("p (s r) -> p s r", r=RSUB[k]),
                )

        def emit_a2a(k):
            # 2-D receiver-major APs: byte-contiguous (HW requirement)
            # with the per-receiver chunk dim explicit
            nc.gpsimd.collective_compute(
                "AllToAll", mybir.AluOpType.bypass,
                replica_groups=GROUP8,
                ins=[cc_in_f[k][:].rearrange("s t p r -> s (t p r)")],
                outs=[cc_out_f[k][:].rearrange("s t p r -> s (t p r)")],
            )

        proj_mms = {}

        def proj_units(k):
            # saTg[p, fc, rcol]: fc = gs*2 + t (feature chunk),
            # rcol = bs*sub + r over both batches = 2*sub columns
            sub = RSUB[k]
            saTg = saTg_pool.tile([P, DC, 2 * max(RSUB)], BF16, tag="saTg",
                                  name="saTg")[:, :, :2 * sub]

            def load_unit():
                for bs in range(2):
                    nc.sync.dma_start(
                        saTg[:, :, bs * sub:(bs + 1) * sub].rearrange(
                            "p (gs t) r -> p gs t r", t=2),
                        cc_out[k][4 * bs:4 * bs + 4, :].rearrange(
                            "gs t p r -> p gs t r"),
                    )

            def mm_unit(s, half):
                rows = min(P, 2 * sub - s * P)
                pps = kqvps.tile([P, 512], F32, tag="kqv", name="ps_proj")
                hsl = slice(half * 512, (half + 1) * 512)
                for f in range(DC):
                    nc.tensor.matmul(
                        pps[:rows],
                        lhsT=saTg[:, f, s * P:s * P + rows],
                        rhs=wpt[:, f, hsl],
                        start=(f == 0), stop=(f == DC - 1),
                    )
                ost = ost_pool.tile([P, 512], F32, tag="ost", name="ost")
                nc.vector.tensor_tensor(
                    out=ost[:rows], in0=pps[:rows],
                    in1=bproj[:rows, hsl], op=mybir.AluOpType.add,
                )
                nc.sync.dma_start(
                    out[OBASE[k] + s * P:OBASE[k] + s * P + rows, hsl],
                    ost[:rows],
                )

            def mm_unit_q(s, qtr):
                # 256-col accumulation group: shorter bias->out tail chain
                pps = kqvps.tile([P, 512], F32, tag="kqv", name="ps_projq")
                rows = min(P, 2 * sub - s * P)
                qsl = slice(qtr * NB, (qtr + 1) * NB)
                for f in range(DC):
                    nc.tensor.matmul(
                        pps[:rows, 0:NB],
                        lhsT=saTg[:, f, s * P:s * P + rows],
                        rhs=wpt[:, f, qsl],
                        start=(f == 0), stop=(f == DC - 1),
                    )
                ost = ost_pool.tile([P, 512], F32, tag="ost",
                                    name="ost")[:, 0:NB]
                nc.vector.tensor_tensor(
                    out=ost[:rows], in0=pps[:rows, 0:NB],
                    in1=bproj[:rows, qsl], op=mybir.AluOpType.add,
                )
                nc.sync.dma_start(
                    out[OBASE[k] + s * P:OBASE[k] + s * P + rows, qsl],
                    ost[:rows],
                )

            units = []
            n_strip = (2 * sub + P - 1) // P
            for s in range(n_strip):
                if k == len(RANGES) - 1:
                    for qtr in range(4):
                        units.append(lambda s=s, qtr=qtr: mm_unit_q(s, qtr))
                else:
                    for half in range(2):
                        units.append(lambda s=s, half=half: mm_unit(s, half))
            return load_unit, units

        def emit_proj_load(k):
            load, mms = proj_units(k)
            load()
            proj_mms[k] = mms

        def proj_filler_units(k):
            return proj_mms.pop(k)

        def emit_proj_mms(k):
            for u in proj_mms.pop(k):
                u()

        # ---------------- emission order ----------------
        emit_consts_a()
        emit_warm(24, 36)
        emit_x_load(0, split=True)
        emit_consts_a2()
        emit_consts_b()
        emit_x_load(1)
        emit_x_load(2)
        emit_x_load(3)
        for u in kqv_units(0):
            u()
        emit_consts_c()
        mark1 = enqueue_fillers(kqv_units(1))
        tick_n[0] = 2
        emit_attention_block(0)
        mark2 = enqueue_fillers(kqv_units(2))
        emit_attention_block(1)
        tick_n[0] = 1
        flush_through(mark1)     # kqv(1) must complete before attn(2)
        emit_attention_block(2)
        mark3 = enqueue_fillers(kqv_units(3, mbs=[12, 13]))
        emit_attention_block(3)
        flush_through(mark2)     # kqv(2) must complete before attn(4)
        emit_wpt()
        emit_attention_block(4)
        emit_attention_block(5)
        flush_through(mark3)     # KQ(3)+V(12,13) must precede attn(6)
        emit_a2a_stage(0)
        emit_a2a(0)              # rows 0:1536 overlap attn(6)+attn(7)
        enqueue_fillers(kqv_units(3, mbs=[14, 15], kq=False))
        emit_attention_block(6)
        flush_fillers()          # V(14,15) must precede attn(7)
        emit_attention_block(7)
        # stage+fire a2a(1) FIRST: any load emitted earlier would hold the
        # SP sequencer through its wait and delay this staging
        emit_a2a_stage(1)
        emit_a2a(1)              # rows 1536:2048
        # bridge the a2a(0) spill window so proj(0) dispatches warm,
        # then proj(0) + a second bridge fill the a2a(1) window
        emit_warm(0, 26, bcols=512)
        emit_proj_load(0)
        emit_proj_mms(0)
        emit_warm(0, 40, bcols=512)
        emit_proj_load(1)
        emit_proj_mms(1)


def build_nc():
    nc = bacc.Bacc(
        "TRN2", target_bir_lowering=False, debug=False,
        num_devices=N_CORES, enable_asserts=False,
    )
    with tile.TileContext(nc) as tc:
        import contextlib
        with contextlib.ExitStack() as ctx:
            build_kernel(tc, ctx)
    nc.finalize()
    return nc


def make_in_maps(x, W_kqv, b_kqv, W_proj, b_proj):
    """Host-side shard + bf16 cast + layout packing (no math beyond rounding)."""
    in_maps = []
    # wpt[p, f, j] = W_proj[j, f*128+p]
    wpt = np.ascontiguousarray(
        np.asarray(W_proj, np.float32).T.reshape(DC, P, D).transpose(1, 0, 2)
    ).astype(BF16_NP)
    bp_rep = np.ascontiguousarray(
        np.broadcast_to(np.asarray(b_proj, np.float32)[None, :], (P, D)))
    for c in range(N_CORES):
        b = c // 4
        g = c % 4
        wl = np.ascontiguousarray(W_kqv[4 * g:4 * g + 4], np.float32)
        bl = np.ascontiguousarray(b_kqv[4 * g:4 * g + 4], np.float32)
        # [p, l, dc, e] view of the per-head weights (l = local head)
        wr = wl.reshape(HPC, DC, P, 3 * HD).transpose(2, 0, 1, 3)
        # wk2/wq2: [p, pr, dc, h2*64+e]
        wk2 = np.ascontiguousarray(
            wr[:, :, :, 0:HD].reshape(P, 2, 2, DC, HD).transpose(0, 1, 3, 2, 4)
            .reshape(P, 2, DC, P)).astype(BF16_NP)
        wq2 = np.ascontiguousarray(
            wr[:, :, :, HD:2 * HD].reshape(P, 2, 2, DC, HD).transpose(0, 1, 3, 2, 4)
            .reshape(P, 2, DC, P)).astype(BF16_NP)
        # wv: [p, dc, l*64+e]
        wv = np.ascontiguousarray(
            wr[:, :, :, 2 * HD:3 * HD].transpose(0, 2, 1, 3)
            .reshape(P, DC, HPC * HD)).astype(BF16_NP)
        # bkq: [64*h2+e, pr, {k,q}]
        bkq = np.zeros((P, 2, 2), np.float32)
        for pr in range(2):
            for h2 in range(2):
                l = 2 * pr + h2
                bkq[64 * h2:64 * h2 + 64, pr, 0] = bl[l, 0:HD]
                bkq[64 * h2:64 * h2 + 64, pr, 1] = bl[l, HD:2 * HD]
        vbias = np.ascontiguousarray(
            np.broadcast_to(bl[:, 2 * HD:3 * HD].reshape(1, HPC * HD),
                            (P, HPC * HD)))
        # xT: [p, dc, n] = x[b, n, dc*128+p]
        xt = np.ascontiguousarray(
            np.asarray(x[b], np.float32).T.reshape(DC, P, N).transpose(1, 0, 2))
        in_maps.append({
            "xtb": xt.astype(BF16_NP),
            "wk2": wk2,
            "wq2": wq2,
            "wv": wv,
            "bkq": bkq,
            "vbias": vbias,
            "bproj": bp_rep,
            "wpt": wpt,
        })
    return in_maps


def assemble(results):
    full = np.zeros((2, N, D), dtype=np.float32)
    for c in range(N_CORES):
        o = results[c]["out"]
        for k in range(len(RANGES)):
            sub = RSUB[k]
            r0 = RBASE[k] + sub * c
            for b in range(2):
                full[b, r0:r0 + sub, :] = \
                    o[OBASE[k] + sub * b:OBASE[k] + sub * (b + 1), :]
    return full


def kernel(x, W_kqv, b_kqv, W_proj, b_proj):
    x = np.asarray(x)
    W_kqv = np.asarray(W_kqv)
    b_kqv = np.asarray(b_kqv)
    W_proj = np.asarray(W_proj)
    b_proj = np.asarray(b_proj)
    nc = build_nc()
    in_maps = make_in_maps(x, W_kqv, b_kqv, W_proj, b_proj)
    res = run_bass_kernel_spmd(nc, in_maps, list(range(N_CORES)))
    return assemble(res.results)


if __name__ == "__main__":
    rng = np.random.default_rng(0)
    x = rng.standard_normal((2, N, D), dtype=np.float32)
    W_kqv = rng.standard_normal((H, D, 3 * HD), dtype=np.float32) / 32
    b_kqv = rng.standard_normal((H, 3 * HD), dtype=np.float32) / 32
    W_proj = rng.standard_normal((D, D), dtype=np.float32) / 32
    b_proj = rng.standard_normal((D,), dtype=np.float32) / 32
    out = kernel(x, W_kqv, b_kqv, W_proj, b_proj)
    print(out.shape, out.dtype, np.abs(out).max())


# revision 52
# speedup vs baseline: 1.0180x; 1.0119x over previous
"""Trainium2 Bass kernel for nn_CausalSelfAttention (B=2, N=2048, D=1024, H=16).

Sharding (8 cores): batch (2-way) x head-group (4-way, 4 heads per core).
Reference swaps K/Q roles: scores = K @ Q^T, softmax over the Q index.

v3 design (all-bf16 compute; fp8 measurably exceeds the 2e-2 gate on the
K/Q path because softmax averaging does not suppress relative weight
jitter):
- Host pre-transposes x and W_proj (no on-device DMA transposes); all x
  quarters stream in up-front while compute proceeds.
- Attention starts right after quarter-0 KQV: strips 0-1 only need the
  first n/m quarter, so the remaining KQV quarters drip into the strip
  pipeline's stall points as fillers (ACT exp and PE matmuls stay
  co-busy from ~4us on).
- ACT runs exp only; K/Q/V bias adds + casts run on DVE straight from
  PSUM (tensor_scalar / tensor_tensor).
- Softmax normalize: DVE reciprocal of the PV ones-row, gpsimd
  partition_broadcast to 64 partitions (SBUF), then one DVE
  tensor_tensor mult PSUM x SBUF -> saT bf16. No PE broadcast matmul.
- AllToAll ranges [1024, 512, 512] fire after strips 3/5/7; the 4-D
  bounce-buffer APs keep the per-shard layout explicit. Projection
  matmuls for ranges 0-1 execute inside the last exchange's window so
  only the 512-row projection trails it.
"""

import sys

import numpy as np

if "/opt/trn_rl_repo" not in sys.path:
    sys.path.insert(0, "/opt/trn_rl_repo")

import ml_dtypes
import concourse.bass as bass
import concourse.mybir as mybir
import concourse.tile as tile
from concourse import bacc
from concourse.bass_utils import run_bass_kernel_spmd

F32 = mybir.dt.float32
BF16 = mybir.dt.bfloat16
BF16_NP = ml_dtypes.bfloat16

P = 128
N = 2048          # sequence length
D = 1024          # model dim
H = 16            # total heads
HPC = 4           # heads per core
HD = 64           # head dim
DC = D // P       # 8 d-chunks
NB = 256          # attention n-block (free dim of S^T tiles)
NBLK = N // NB    # 8 strips
MB = N // P       # 16 m-blocks
CHUNK = 4         # m-blocks per PSUM strip (4*256 fp32 = 2 PSUM banks)
N_CORES = 8

RANGES = [1536, 512]
RBASE = [0, 1536]
RSUB = [r // 8 for r in RANGES]          # rows per core per range
OBASE = [0, 384]                         # output row base per range (per core)

GROUP8 = [[0, 1, 2, 3, 4, 5, 6, 7]]


def _mask_np():
    # causal mask for the diagonal m-block pair of each strip:
    # cols 0:256   (m_blk 2J,   m = 256J + p, n-cols 0:256)   keep j >= p
    # cols 256:384 (m_blk 2J+1, m = 256J+128+p, n-cols 128:256) keep j >= p
    # (the S matmul for block 2J+1 only computes the upper n-half)
    p = np.arange(P)[:, None]
    m0 = (np.arange(256)[None, :] >= p).astype(np.float32)
    m1 = (np.arange(128)[None, :] >= p).astype(np.float32)
    return np.concatenate([m0, m1], axis=1).astype(BF16_NP)


def build_kernel(tc: tile.TileContext, ctx):
    nc = tc.nc

    xtb_ext = nc.dram_tensor("xtb", [P, DC, N], BF16, kind="ExternalInput")
    wk2_ext = nc.dram_tensor("wk2", [P, 2, DC, P], BF16, kind="ExternalInput")
    wq2_ext = nc.dram_tensor("wq2", [P, 2, DC, P], BF16, kind="ExternalInput")
    wv_ext = nc.dram_tensor("wv", [P, DC, HPC * HD], BF16, kind="ExternalInput")
    bkq_ext = nc.dram_tensor("bkq", [P, 2, 2], F32, kind="ExternalInput")
    vbias_ext = nc.dram_tensor("vbias", [P, HPC * HD], F32, kind="ExternalInput")
    bproj_ext = nc.dram_tensor("bproj", [P, D], F32, kind="ExternalInput")
    wpt_ext = nc.dram_tensor("wpt", [P, DC, D], BF16, kind="ExternalInput")
    out_ext = nc.dram_tensor("out", [512, D], F32, kind="ExternalOutput")

    out = out_ext[:]

    mask_dram = nc.inline_tensor(_mask_np(), name="mask_c")

    dram = ctx.enter_context(tc.tile_pool(name="dram", bufs=1, space="DRAM"))
    const = ctx.enter_context(tc.tile_pool(name="const", bufs=1))

    # AllToAll buffers: [8 chunks (receiver), 2 t, 128 p, sub r] bf16.
    # Row-padded so the per-receiver leading dim stays explicit in the
    # lowered access pattern (strided slices don't collapse).
    CPAD = 0
    cc_in_f = [dram.tile([8, 2, P, RSUB[k] + CPAD], BF16, name=f"cc_in{k}")
               for k in range(len(RANGES))]
    cc_out_f = [dram.tile([8, 2, P, RSUB[k] + CPAD], BF16, name=f"cc_out{k}")
                for k in range(len(RANGES))]
    cc_in = [t[:, :, :, 0:RSUB[k]] for k, t in enumerate(cc_in_f)]
    cc_out = [t[:, :, :, 0:RSUB[k]] for k, t in enumerate(cc_out_f)]

    # ---------------- SBUF constants / weights ----------------
    # per-quarter x^T tiles: separate tiles keep the dependency tracker's
    # byte-range bounding boxes quarter-precise (a unified tile serializes
    # the whole pipeline behind the last x quarter's DMA)
    xtb_q = [const.tile([P, DC, N // 4], BF16, name=f"xtb{q}")
             for q in range(4)]
    wk2 = const.tile([P, 2, DC, P], BF16, name="wk2")
    wq2 = const.tile([P, 2, DC, P], BF16, name="wq2")
    wv = const.tile([P, DC, HPC * HD], BF16, name="wv")
    bkq = const.tile([P, 2, 2], F32, name="bkq")
    vbias = const.tile([P, HPC * HD], F32, name="vbias")
    bproj = const.tile([P, D], F32, name="bproj")
    mask = const.tile([P, 384], BF16, name="mask")
    wpt = const.tile([P, DC, D], BF16, name="wpt")

    # PE p-state warmup dummies (memset-initialized, no DMA dependency)
    dw = const.tile([1, 1], BF16, name="dw")
    dx = const.tile([1, 512], BF16, name="dx")

    # ---------------- KQV / attention state ----------------
    # k2/q2: [64*h2+e, pr, n] bf16 (packed head-pair partition layout)
    k2 = const.tile([P, 2, N], BF16, name="k2")
    q2 = const.tile([P, 2, N], BF16, name="q2")
    v = const.tile([P, MB, HPC * (HD + 1)], BF16, name="v")
    saT = const.tile([P, 2, N], BF16, name="saT")

    NQ = N // 4

    def emit_x_load(ns, split=False):
        qsl = slice(ns * NQ, (ns + 1) * NQ)
        if split:
            # dc-halves so the first KQ matmuls start after half the bytes
            nc.sync.dma_start(xtb_q[ns][:, 0:4, :], xtb_ext[:, 0:4, qsl])
            nc.sync.dma_start(xtb_q[ns][:, 4:8, :], xtb_ext[:, 4:8, qsl])
        else:
            nc.sync.dma_start(xtb_q[ns][:], xtb_ext[:, :, qsl])

    def emit_wpt():
        for hh in range(2):
            nc.sync.dma_start(wpt[:, 4 * hh:4 * hh + 4, :],
                              wpt_ext[:, 4 * hh:4 * hh + 4, :])

    with tc.tile_pool(name="kqv_ps", bufs=2, space="PSUM") as kqvps, \
         tc.tile_pool(name="strip_ps", bufs=2, space="PSUM") as strip_ps, \
         tc.tile_pool(name="acc_ps", bufs=2, space="PSUM") as acc_ps, \
         tc.tile_pool(name="pt_pool", bufs=8) as pt_pool, \
         tc.tile_pool(name="small", bufs=3) as small, \
         tc.tile_pool(name="bc_pool", bufs=3) as bc_pool, \
         tc.tile_pool(name="saTg_pool", bufs=2) as saTg_pool, \
         tc.tile_pool(name="ost_pool", bufs=3) as ost_pool:

        def emit_warm(tiny, bridge, bcols=128):
            # tiny 8-col matmuls age the PE p-state ramp (~7ns each); the
            # 512-col bridge matmuls keep the busy-streak alive across a
            # known PE-idle window so real matmuls behind them charge at
            # full clock. All write a throwaway [1, x] PSUM row.
            wps = kqvps.tile([P, 512], F32, tag="kqv", name="ps_warm")
            for _ in range(tiny):
                nc.tensor.matmul(wps[0:1, 0:8], lhsT=dw[:], rhs=dx[0:1, 0:8],
                                 start=True, stop=True)
            for _ in range(bridge):
                nc.tensor.matmul(wps[0:1, 0:bcols], lhsT=dw[:],
                                 rhs=dx[0:1, 0:bcols], start=True, stop=True)

        def emit_consts_a():
            # small weights on the SP/HWDGE path, requested before the big
            # x quarters so the DMA-engine FIFO serves them first
            nc.gpsimd.memset(dw[:], 1.0)
            nc.gpsimd.memset(dx[:], 1.0)
            nc.sync.dma_start(wk2[:, 0], wk2_ext[:, 0])
            nc.sync.dma_start(bkq[:], bkq_ext[:])

        def emit_consts_a2():
            nc.sync.dma_start(wq2[:, 0], wq2_ext[:, 0])
            nc.sync.dma_start(wk2[:, 1], wk2_ext[:, 1])
            nc.sync.dma_start(wq2[:, 1], wq2_ext[:, 1])

        def emit_consts_b():
            nc.sync.dma_start(wv[:], wv_ext[:])
            nc.sync.dma_start(vbias[:], vbias_ext[:])
            nc.sync.dma_start(mask[:], mask_dram[:])

        def emit_consts_c():
            nc.gpsimd.dma_start(bproj[:], bproj_ext[:])
            # ones column per head (denominator row of the PV matmul)
            nc.gpsimd.memset(
                v[:].rearrange("p m (h c) -> p m h c", c=HD + 1)[:, :, :, HD:HD + 1],
                1.0,
            )

        def emit_kq_unit(ns, pr, which):
            nsl = slice(ns * 512, (ns + 1) * 512)
            ps = kqvps.tile([P, 512], F32, tag="kqv", name="ps_kq")
            w = wk2 if which == 0 else wq2
            for dc in range(DC):
                nc.tensor.matmul(
                    ps[:], lhsT=w[:, pr, dc, :], rhs=xtb_q[ns][:, dc, :],
                    start=(dc == 0), stop=(dc == DC - 1),
                )
            dst = k2 if which == 0 else q2
            nc.vector.tensor_scalar(
                out=dst[:, pr, nsl], in0=ps[:],
                scalar1=bkq[:, pr, which:which + 1], scalar2=None,
                op0=mybir.AluOpType.add,
            )

        def emit_v_unit(ns, mb):
            msl = slice((mb % 4) * P, (mb % 4 + 1) * P)
            ps = kqvps.tile([P, 512], F32, tag="kqv", name="ps_v")
            for dc in range(DC):
                nc.tensor.matmul(
                    ps[:, :HPC * HD], lhsT=xtb_q[ns][:, dc, msl],
                    rhs=wv[:, dc, :],
                    start=(dc == 0), stop=(dc == DC - 1),
                )
            nc.vector.tensor_tensor(
                out=v[:].rearrange("p m (h c) -> p m h c", c=HD + 1)[:, mb, :, 0:HD],
                in0=ps[:, :HPC * HD].rearrange("p (h e) -> p h e", e=HD),
                in1=vbias[:].rearrange("p (h e) -> p h e", e=HD),
                op=mybir.AluOpType.add,
            )

        def kqv_units(ns, mbs=None, kq=True):
            units = []
            if kq:
                for pr in range(2):
                    for which in range(2):
                        units.append(lambda ns=ns, pr=pr, which=which:
                                     emit_kq_unit(ns, pr, which))
            if mbs is None:
                mbs = range(4 * ns, 4 * ns + 4)
            for mb in mbs:
                units.append(lambda ns=ns, mb=mb: emit_v_unit(ns, mb))
            return units

        # filler machinery: KQV work units dripped into the attention stream
        filler_q = []
        fill_stat = {"queued": 0, "popped": 0}
        tick_n = [1]

        def enqueue_fillers(units):
            filler_q.extend(units)
            fill_stat["queued"] += len(units)
            return fill_stat["queued"]

        def round_tick():
            for _ in range(tick_n[0]):
                if filler_q:
                    filler_q.pop(0)()
                    fill_stat["popped"] += 1

        def flush_through(target):
            while fill_stat["popped"] < target and filler_q:
                filler_q.pop(0)()
                fill_stat["popped"] += 1

        def flush_fillers():
            while filler_q:
                filler_q.pop(0)()
                fill_stat["popped"] += 1

        def emit_attention_block(J):
            """Two heads of each partition-pair in lockstep; PV lags the
            S/exp pipeline so ACT overlaps PE."""
            nsl = slice(J * NB, (J + 1) * NB)
            n_mb = 2 * (J + 1)
            for pr in range(2):
                heads = []
                for h2 in range(2):
                    # one PSUM bank per head; den row at partition 64. Late
                    # blocks borrow the idle kqv pool for pr=1 so the second
                    # pair never waits on the first pair's bank release.
                    if J >= 6 and pr == 1:
                        bank = kqvps.tile([P, 512], F32, tag="kqv",
                                          name=f"ps_acc{h2}")
                    else:
                        bank = acc_ps.tile([P, 512], F32, tag="acc",
                                           name=f"ps_acc{h2}")
                    heads.append({"h2": h2, "l": 2 * pr + h2,
                                  "prow": slice(HD * h2, HD * h2 + HD),
                                  "opsf": bank[:, 0:NB]})
                pending = []  # (head, c0, cn, pts)

                def emit_s(hd, c0, cn):
                    # the last chunk holds the diagonal pair; block 2J+1
                    # only computes (and exps) its valid upper n-half
                    has_diag = c0 <= 2 * J < c0 + cn
                    wid = cn * NB - (128 if has_diag else 0)
                    sps = strip_ps.tile(
                        [P, CHUNK * NB], F32, tag="strip", name="ps_strip"
                    )[:, :wid]
                    for a in range(c0, c0 + cn):
                        o = (a - c0) * NB
                        if has_diag and a == 2 * J + 1:
                            nc.tensor.matmul(
                                sps[:, o:o + 128],
                                lhsT=q2[hd["prow"], pr, a * P:(a + 1) * P],
                                rhs=k2[hd["prow"], pr,
                                       J * NB + 128:(J + 1) * NB],
                                start=True, stop=True,
                            )
                        else:
                            nc.tensor.matmul(
                                sps[:, o:o + NB],
                                lhsT=q2[hd["prow"], pr, a * P:(a + 1) * P],
                                rhs=k2[hd["prow"], pr, nsl],
                                start=True, stop=True,
                            )
                    pts = pt_pool.tile(
                        [P, CHUNK * NB], BF16, tag="pt", name="pt"
                    )[:, :wid]
                    nc.scalar.activation(
                        pts, sps, mybir.ActivationFunctionType.Exp,
                        scale=1.0 / np.sqrt(HD),
                    )
                    if has_diag:
                        o = (2 * J - c0) * NB
                        nc.vector.tensor_tensor(
                            out=pts[:, o:o + 384], in0=pts[:, o:o + 384],
                            in1=mask[:], op=mybir.AluOpType.mult,
                        )
                    pending.append((hd, c0, cn, pts, has_diag))

                def emit_pv(hd, c0, cn, pts, has_diag):
                    lcol = hd["l"] * (HD + 1)
                    for a in range(c0, c0 + cn):
                        o = (a - c0) * NB
                        if has_diag and a == 2 * J + 1:
                            nc.tensor.matmul(
                                hd["opsf"][0:HD + 1, 128:NB],
                                lhsT=v[:, a, lcol:lcol + HD + 1],
                                rhs=pts[:, o:o + 128],
                                start=False, stop=(a == n_mb - 1),
                            )
                        else:
                            nc.tensor.matmul(
                                hd["opsf"][0:HD + 1, :],
                                lhsT=v[:, a, lcol:lcol + HD + 1],
                                rhs=pts[:, o:o + NB],
                                start=(a == 0), stop=(a == n_mb - 1),
                            )

                first = True
                for c0 in range(0, n_mb, CHUNK):
                    cn = min(CHUNK, n_mb - c0)
                    if not first:
                        round_tick()
                    emit_s(heads[0], c0, cn)
                    emit_s(heads[1], c0, cn)
                    while len(pending) > 5:
                        emit_pv(*pending.pop(0))
                    first = False
                while pending:
                    emit_pv(*pending.pop(0))

                # finalize: rc = 1/den, gpsimd broadcast to 64 partitions
                # (SBUF), one normalize-mult PSUM x SBUF -> saT bf16.
                for hd in heads:
                    round_tick()
                    h2 = hd["h2"]
                    rc = small.tile([1, NB], F32, tag="rc", name="rc")
                    nc.vector.reciprocal(rc[:], hd["opsf"][HD:HD + 1, :])
                    bc = bc_pool.tile([HD, NB], F32, tag="bc", name="bc")
                    nc.gpsimd.partition_broadcast(bc[:], rc[:], channels=HD)
                    nc.vector.tensor_tensor(
                        out=saT[hd["prow"], pr, nsl],
                        in0=hd["opsf"][0:HD, :],
                        in1=bc[:],
                        op=mybir.AluOpType.mult,
                    )

        def emit_a2a_stage(k):
            # t=0 (pr0, finalizes first) and t=1 on separate engine queues
            # so the two staging DMAs overlap
            nsl = slice(RBASE[k], RBASE[k] + RANGES[k])
            for t, eng in ((0, nc.sync), (1, nc.sync)):
                eng.dma_start(
                    cc_in[k][:, t].rearrange("s p r -> p s r"),
                    saT[:, t, nsl].rearrange("p (s r) -> p s r", r=RSUB[k]),
                )

        def emit_a2a(k):
            # 2-D receiver-major APs: byte-contiguous (HW requirement)
            # with the per-receiver chunk dim explicit
            nc.gpsimd.collective_compute(
                "AllToAll", mybir.AluOpType.bypass,
                replica_groups=GROUP8,
                ins=[cc_in_f[k][:].rearrange("s t p r -> s (t p r)")],
                outs=[cc_out_f[k][:].rearrange("s t p r -> s (t p r)")],
            )

        proj_mms = {}

        def proj_units(k):
            # saTg[p, fc, rcol]: fc = gs*2 + t (feature chunk),
            # rcol = bs*sub + r over both batches = 2*sub columns
            sub = RSUB[k]
            saTg = saTg_pool.tile([P, DC, 2 * max(RSUB)], BF16, tag="saTg",
                                  name="saTg")[:, :, :2 * sub]

            def load_unit():
                for bs in range(2):
                    nc.sync.dma_start(
                        saTg[:, :, bs * sub:(bs + 1) * sub].rearrange(
                            "p (gs t) r -> p gs t r", t=2),
                        cc_out[k][4 * bs:4 * bs + 4, :].rearrange(
                            "gs t p r -> p gs t r"),
                    )

            def mm_unit(s, half):
                rows = min(P, 2 * sub - s * P)
                pps = kqvps.tile([P, 512], F32, tag="kqv", name="ps_proj")
                hsl = slice(half * 512, (half + 1) * 512)
                for f in range(DC):
                    nc.tensor.matmul(
                        pps[:rows],
                        lhsT=saTg[:, f, s * P:s * P + rows],
                        rhs=wpt[:, f, hsl],
                        start=(f == 0), stop=(f == DC - 1),
                    )
                ost = ost_pool.tile([P, 512], F32, tag="ost", name="ost")
                nc.vector.tensor_tensor(
                    out=ost[:rows], in0=pps[:rows],
                    in1=bproj[:rows, hsl], op=mybir.AluOpType.add,
                )
                nc.sync.dma_start(
                    out[OBASE[k] + s * P:OBASE[k] + s * P + rows, hsl],
                    ost[:rows],
                )

            def mm_unit_q(s, qtr):
                # 256-col accumulation group: shorter bias->out tail chain
                pps = kqvps.tile([P, 512], F32, tag="kqv", name="ps_projq")
                rows = min(P, 2 * sub - s * P)
                qsl = slice(qtr * NB, (qtr + 1) * NB)
                for f in range(DC):
                    nc.tensor.matmul(
                        pps[:rows, 0:NB],
                        lhsT=saTg[:, f, s * P:s * P + rows],
                        rhs=wpt[:, f, qsl],
                        start=(f == 0), stop=(f == DC - 1),
                    )
                ost = ost_pool.tile([P, 512], F32, tag="ost",
                                    name="ost")[:, 0:NB]
                nc.vector.tensor_tensor(
                    out=ost[:rows], in0=pps[:rows, 0:NB],
                    in1=bproj[:rows, qsl], op=mybir.AluOpType.add,
                )
                nc.sync.dma_start(
                    out[OBASE[k] + s * P:OBASE[k] + s * P + rows, qsl],
                    ost[:rows],
                )

            units = []
            n_strip = (2 * sub + P - 1) // P
            for s in range(n_strip):
                if k == len(RANGES) - 1:
                    for qtr in range(4):
                        units.append(lambda s=s, qtr=qtr: mm_unit_q(s, qtr))
                else:
                    for half in range(2):
                        units.append(lambda s=s, half=half: mm_unit(s, half))
            return load_unit, units

        def emit_proj_load(k):
            load, mms = proj_units(k)
            load()
            proj_mms[k] = mms

        def proj_filler_units(k):
            return proj_mms.pop(k)

        def emit_proj_mms(k):
            for u in proj_mms.pop(k):
                u()

        # ---------------- emission order ----------------
        emit_consts_a()
        emit_warm(24, 36)
        emit_x_load(0, split=True)
        emit_consts_a2()
        emit_consts_b()
        emit_x_load(1)
        emit_x_load(2)
        emit_x_load(3)
        for u in kqv_units(0):
            u()
        emit_consts_c()
        mark1 = enqueue_fillers(kqv_units(1))
        tick_n[0] = 2
        emit_attention_block(0)
        mark2 = enqueue_fillers(kqv_units(2))
        emit_attention_block(1)
        tick_n[0] = 1
        flush_through(mark1)     # kqv(1) must complete before attn(2)
        emit_attention_block(2)
        mark3 = enqueue_fillers(kqv_units(3, mbs=[12, 13]))
        emit_attention_block(3)
        flush_through(mark2)     # kqv(2) must complete before attn(4)
        emit_wpt()
        emit_attention_block(4)
        emit_attention_block(5)
        flush_through(mark3)     # KQ(3)+V(12,13) must precede attn(6)
        emit_a2a_stage(0)
        emit_a2a(0)              # rows 0:1536 overlap attn(6)+attn(7)
        enqueue_fillers(kqv_units(3, mbs=[14, 15], kq=False))
        emit_attention_block(6)
        flush_fillers()          # V(14,15) must precede attn(7)
        emit_attention_block(7)
        # stage+fire a2a(1) FIRST: any load emitted earlier would hold the
        # SP sequencer through its wait and delay this staging
        emit_a2a_stage(1)
        emit_a2a(1)              # rows 1536:2048
        # bridge the a2a(0) spill window so proj(0) dispatches warm,
        # then proj(0) + a second bridge fill the a2a(1) window
        emit_warm(0, 26, bcols=512)
        emit_proj_load(0)
        emit_proj_mms(0)
        emit_warm(0, 40, bcols=512)
        emit_proj_load(1)
        emit_proj_mms(1)


def build_nc():
    nc = bacc.Bacc(
        "TRN2", target_bir_lowering=False, debug=False,
        num_devices=N_CORES, enable_asserts=False,
    )
    with tile.TileContext(nc) as tc:
        import contextlib
        with contextlib.ExitStack() as ctx:
            build_kernel(tc, ctx)
    nc.finalize()
    return nc


def make_in_maps(x, W_kqv, b_kqv, W_proj, b_proj):
    """Host-side shard + bf16 cast + layout packing (no math beyond rounding)."""
    in_maps = []
    # wpt[p, f, j] = W_proj[j, f*128+p]
    wpt = np.ascontiguousarray(
        np.asarray(W_proj, np.float32).T.reshape(DC, P, D).transpose(1, 0, 2)
    ).astype(BF16_NP)
    bp_rep = np.ascontiguousarray(
        np.broadcast_to(np.asarray(b_proj, np.float32)[None, :], (P, D)))
    for c in range(N_CORES):
        b = c // 4
        g = c % 4
        wl = np.ascontiguousarray(W_kqv[4 * g:4 * g + 4], np.float32)
        bl = np.ascontiguousarray(b_kqv[4 * g:4 * g + 4], np.float32)
        # [p, l, dc, e] view of the per-head weights (l = local head)
        wr = wl.reshape(HPC, DC, P, 3 * HD).transpose(2, 0, 1, 3)
        # wk2/wq2: [p, pr, dc, h2*64+e]
        wk2 = np.ascontiguousarray(
            wr[:, :, :, 0:HD].reshape(P, 2, 2, DC, HD).transpose(0, 1, 3, 2, 4)
            .reshape(P, 2, DC, P)).astype(BF16_NP)
        wq2 = np.ascontiguousarray(
            wr[:, :, :, HD:2 * HD].reshape(P, 2, 2, DC, HD).transpose(0, 1, 3, 2, 4)
            .reshape(P, 2, DC, P)).astype(BF16_NP)
        # wv: [p, dc, l*64+e]
        wv = np.ascontiguousarray(
            wr[:, :, :, 2 * HD:3 * HD].transpose(0, 2, 1, 3)
            .reshape(P, DC, HPC * HD)).astype(BF16_NP)
        # bkq: [64*h2+e, pr, {k,q}]
        bkq = np.zeros((P, 2, 2), np.float32)
        for pr in range(2):
            for h2 in range(2):
                l = 2 * pr + h2
                bkq[64 * h2:64 * h2 + 64, pr, 0] = bl[l, 0:HD]
                bkq[64 * h2:64 * h2 + 64, pr, 1] = bl[l, HD:2 * HD]
        vbias = np.ascontiguousarray(
            np.broadcast_to(bl[:, 2 * HD:3 * HD].reshape(1, HPC * HD),
                            (P, HPC * HD)))
        # xT: [p, dc, n] = x[b, n, dc*128+p]
        xt = np.ascontiguousarray(
            np.asarray(x[b], np.float32).T.reshape(DC, P, N).transpose(1, 0, 2))
        in_maps.append({
            "xtb": xt.astype(BF16_NP),
            "wk2": wk2,
            "wq2": wq2,
            "wv": wv,
            "bkq": bkq,
            "vbias": vbias,
            "bproj": bp_rep,
            "wpt": wpt,
        })
    return in_maps


def assemble(results):
    full = np.zeros((2, N, D), dtype=np.float32)
    for c in range(N_CORES):
        o = results[c]["out"]
        for k in range(len(RANGES)):
            sub = RSUB[k]
            r0 = RBASE[k] + sub * c
            for b in range(2):
                full[b, r0:r0 + sub, :] = \
                    o[OBASE[k] + sub * b:OBASE[k] + sub * (b + 1), :]
    return full


def kernel(x, W_kqv, b_kqv, W_proj, b_proj):
    x = np.asarray(x)
    W_kqv = np.asarray(W_kqv)
    b_kqv = np.asarray(b_kqv)
    W_proj = np.asarray(W_proj)
    b_proj = np.asarray(b_proj)
    nc = build_nc()
    in_maps = make_in_maps(x, W_kqv, b_kqv, W_proj, b_proj)
    res = run_bass_kernel_spmd(nc, in_maps, list(range(N_CORES)))
    return assemble(res.results)


if __name__ == "__main__":
    rng = np.random.default_rng(0)
    x = rng.standard_normal((2, N, D), dtype=np.float32)
    W_kqv = rng.standard_normal((H, D, 3 * HD), dtype=np.float32) / 32
    b_kqv = rng.standard_normal((H, 3 * HD), dtype=np.float32) / 32
    W_proj = rng.standard_normal((D, D), dtype=np.float32) / 32
    b_proj = rng.standard_normal((D,), dtype=np.float32) / 32
    out = kernel(x, W_kqv, b_kqv, W_proj, b_proj)
    print(out.shape, out.dtype, np.abs(out).max())


# revision 53
# speedup vs baseline: 1.0192x; 1.0012x over previous
"""Trainium2 Bass kernel for nn_CausalSelfAttention (B=2, N=2048, D=1024, H=16).

Sharding (8 cores): batch (2-way) x head-group (4-way, 4 heads per core).
Reference swaps K/Q roles: scores = K @ Q^T, softmax over the Q index.

v3 design (all-bf16 compute; fp8 measurably exceeds the 2e-2 gate on the
K/Q path because softmax averaging does not suppress relative weight
jitter):
- Host pre-transposes x and W_proj (no on-device DMA transposes); all x
  quarters stream in up-front while compute proceeds.
- Attention starts right after quarter-0 KQV: strips 0-1 only need the
  first n/m quarter, so the remaining KQV quarters drip into the strip
  pipeline's stall points as fillers (ACT exp and PE matmuls stay
  co-busy from ~4us on).
- ACT runs exp only; K/Q/V bias adds + casts run on DVE straight from
  PSUM (tensor_scalar / tensor_tensor).
- Softmax normalize: DVE reciprocal of the PV ones-row, gpsimd
  partition_broadcast to 64 partitions (SBUF), then one DVE
  tensor_tensor mult PSUM x SBUF -> saT bf16. No PE broadcast matmul.
- AllToAll ranges [1024, 512, 512] fire after strips 3/5/7; the 4-D
  bounce-buffer APs keep the per-shard layout explicit. Projection
  matmuls for ranges 0-1 execute inside the last exchange's window so
  only the 512-row projection trails it.
"""

import sys

import numpy as np

if "/opt/trn_rl_repo" not in sys.path:
    sys.path.insert(0, "/opt/trn_rl_repo")

import ml_dtypes
import concourse.bass as bass
import concourse.mybir as mybir
import concourse.tile as tile
from concourse import bacc
from concourse.bass_utils import run_bass_kernel_spmd

F32 = mybir.dt.float32
BF16 = mybir.dt.bfloat16
BF16_NP = ml_dtypes.bfloat16

P = 128
N = 2048          # sequence length
D = 1024          # model dim
H = 16            # total heads
HPC = 4           # heads per core
HD = 64           # head dim
DC = D // P       # 8 d-chunks
NB = 256          # attention n-block (free dim of S^T tiles)
NBLK = N // NB    # 8 strips
MB = N // P       # 16 m-blocks
CHUNK = 4         # m-blocks per PSUM strip (4*256 fp32 = 2 PSUM banks)
N_CORES = 8

RANGES = [1536, 512]
RBASE = [0, 1536]
RSUB = [r // 8 for r in RANGES]          # rows per core per range
OBASE = [0, 384]                         # output row base per range (per core)

GROUP8 = [[0, 1, 2, 3, 4, 5, 6, 7]]


def _mask_np():
    # causal mask for the diagonal m-block pair of each strip:
    # cols 0:256   (m_blk 2J,   m = 256J + p, n-cols 0:256)   keep j >= p
    # cols 256:384 (m_blk 2J+1, m = 256J+128+p, n-cols 128:256) keep j >= p
    # (the S matmul for block 2J+1 only computes the upper n-half)
    p = np.arange(P)[:, None]
    m0 = (np.arange(256)[None, :] >= p).astype(np.float32)
    m1 = (np.arange(128)[None, :] >= p).astype(np.float32)
    return np.concatenate([m0, m1], axis=1).astype(BF16_NP)


def build_kernel(tc: tile.TileContext, ctx):
    nc = tc.nc

    xtb_ext = nc.dram_tensor("xtb", [P, DC, N], BF16, kind="ExternalInput")
    wk2_ext = nc.dram_tensor("wk2", [P, 2, DC, P], BF16, kind="ExternalInput")
    wq2_ext = nc.dram_tensor("wq2", [P, 2, DC, P], BF16, kind="ExternalInput")
    wv_ext = nc.dram_tensor("wv", [P, DC, HPC * HD], BF16, kind="ExternalInput")
    bkq_ext = nc.dram_tensor("bkq", [P, 2, 2], F32, kind="ExternalInput")
    vbias_ext = nc.dram_tensor("vbias", [P, HPC * HD], F32, kind="ExternalInput")
    bproj_ext = nc.dram_tensor("bproj", [P, D], F32, kind="ExternalInput")
    wpt_ext = nc.dram_tensor("wpt", [P, DC, D], BF16, kind="ExternalInput")
    out_ext = nc.dram_tensor("out", [512, D], F32, kind="ExternalOutput")

    out = out_ext[:]

    mask_dram = nc.inline_tensor(_mask_np(), name="mask_c")

    dram = ctx.enter_context(tc.tile_pool(name="dram", bufs=1, space="DRAM"))
    const = ctx.enter_context(tc.tile_pool(name="const", bufs=1))

    # AllToAll buffers: [8 chunks (receiver), 2 t, 128 p, sub r] bf16.
    # Row-padded so the per-receiver leading dim stays explicit in the
    # lowered access pattern (strided slices don't collapse).
    CPAD = 0
    cc_in_f = [dram.tile([8, 2, P, RSUB[k] + CPAD], BF16, name=f"cc_in{k}")
               for k in range(len(RANGES))]
    cc_out_f = [dram.tile([8, 2, P, RSUB[k] + CPAD], BF16, name=f"cc_out{k}")
                for k in range(len(RANGES))]
    cc_in = [t[:, :, :, 0:RSUB[k]] for k, t in enumerate(cc_in_f)]
    cc_out = [t[:, :, :, 0:RSUB[k]] for k, t in enumerate(cc_out_f)]

    # ---------------- SBUF constants / weights ----------------
    # per-quarter x^T tiles: separate tiles keep the dependency tracker's
    # byte-range bounding boxes quarter-precise (a unified tile serializes
    # the whole pipeline behind the last x quarter's DMA)
    xtb_q = [const.tile([P, DC, N // 4], BF16, name=f"xtb{q}")
             for q in range(4)]
    wk2 = const.tile([P, 2, DC, P], BF16, name="wk2")
    wq2 = const.tile([P, 2, DC, P], BF16, name="wq2")
    wv = const.tile([P, DC, HPC * HD], BF16, name="wv")
    bkq = const.tile([P, 2, 2], F32, name="bkq")
    vbias = const.tile([P, HPC * HD], F32, name="vbias")
    bproj = const.tile([P, D], F32, name="bproj")
    mask = const.tile([P, 384], BF16, name="mask")
    wpt = const.tile([P, DC, D], BF16, name="wpt")

    # PE p-state warmup dummies (memset-initialized, no DMA dependency)
    dw = const.tile([1, 1], BF16, name="dw")
    dx = const.tile([1, 512], BF16, name="dx")

    # ---------------- KQV / attention state ----------------
    # k2/q2: [64*h2+e, pr, n] bf16 (packed head-pair partition layout)
    k2 = const.tile([P, 2, N], BF16, name="k2")
    q2 = const.tile([P, 2, N], BF16, name="q2")
    v = const.tile([P, MB, HPC * (HD + 1)], BF16, name="v")
    saT = const.tile([P, 2, N], BF16, name="saT")

    NQ = N // 4

    def emit_x_load(ns, split=False):
        qsl = slice(ns * NQ, (ns + 1) * NQ)
        if split:
            # dc-halves so the first KQ matmuls start after half the bytes
            nc.sync.dma_start(xtb_q[ns][:, 0:4, :], xtb_ext[:, 0:4, qsl])
            nc.sync.dma_start(xtb_q[ns][:, 4:8, :], xtb_ext[:, 4:8, qsl])
        else:
            nc.sync.dma_start(xtb_q[ns][:], xtb_ext[:, :, qsl])

    def emit_wpt():
        for hh in range(2):
            nc.sync.dma_start(wpt[:, 4 * hh:4 * hh + 4, :],
                              wpt_ext[:, 4 * hh:4 * hh + 4, :])

    with tc.tile_pool(name="kqv_ps", bufs=2, space="PSUM") as kqvps, \
         tc.tile_pool(name="strip_ps", bufs=2, space="PSUM") as strip_ps, \
         tc.tile_pool(name="acc_ps", bufs=2, space="PSUM") as acc_ps, \
         tc.tile_pool(name="pt_pool", bufs=10) as pt_pool, \
         tc.tile_pool(name="small", bufs=3) as small, \
         tc.tile_pool(name="bc_pool", bufs=3) as bc_pool, \
         tc.tile_pool(name="saTg_pool", bufs=2) as saTg_pool, \
         tc.tile_pool(name="ost_pool", bufs=3) as ost_pool:

        def emit_warm(tiny, bridge, bcols=128):
            # tiny 8-col matmuls age the PE p-state ramp (~7ns each); the
            # 512-col bridge matmuls keep the busy-streak alive across a
            # known PE-idle window so real matmuls behind them charge at
            # full clock. All write a throwaway [1, x] PSUM row.
            wps = kqvps.tile([P, 512], F32, tag="kqv", name="ps_warm")
            for _ in range(tiny):
                nc.tensor.matmul(wps[0:1, 0:8], lhsT=dw[:], rhs=dx[0:1, 0:8],
                                 start=True, stop=True)
            for _ in range(bridge):
                nc.tensor.matmul(wps[0:1, 0:bcols], lhsT=dw[:],
                                 rhs=dx[0:1, 0:bcols], start=True, stop=True)

        def emit_consts_a():
            # small weights on the SP/HWDGE path, requested before the big
            # x quarters so the DMA-engine FIFO serves them first
            nc.gpsimd.memset(dw[:], 1.0)
            nc.gpsimd.memset(dx[:], 1.0)
            nc.sync.dma_start(wk2[:, 0], wk2_ext[:, 0])
            nc.sync.dma_start(bkq[:], bkq_ext[:])

        def emit_consts_a2():
            nc.sync.dma_start(wq2[:, 0], wq2_ext[:, 0])
            nc.sync.dma_start(wk2[:, 1], wk2_ext[:, 1])
            nc.sync.dma_start(wq2[:, 1], wq2_ext[:, 1])

        def emit_consts_b():
            nc.sync.dma_start(wv[:], wv_ext[:])
            nc.sync.dma_start(vbias[:], vbias_ext[:])
            nc.sync.dma_start(mask[:], mask_dram[:])

        def emit_consts_c():
            nc.gpsimd.dma_start(bproj[:], bproj_ext[:])
            # ones column per head (denominator row of the PV matmul)
            nc.gpsimd.memset(
                v[:].rearrange("p m (h c) -> p m h c", c=HD + 1)[:, :, :, HD:HD + 1],
                1.0,
            )

        def emit_kq_unit(ns, pr, which):
            nsl = slice(ns * 512, (ns + 1) * 512)
            ps = kqvps.tile([P, 512], F32, tag="kqv", name="ps_kq")
            w = wk2 if which == 0 else wq2
            for dc in range(DC):
                nc.tensor.matmul(
                    ps[:], lhsT=w[:, pr, dc, :], rhs=xtb_q[ns][:, dc, :],
                    start=(dc == 0), stop=(dc == DC - 1),
                )
            dst = k2 if which == 0 else q2
            nc.vector.tensor_scalar(
                out=dst[:, pr, nsl], in0=ps[:],
                scalar1=bkq[:, pr, which:which + 1], scalar2=None,
                op0=mybir.AluOpType.add,
            )

        def emit_v_unit(ns, mb):
            msl = slice((mb % 4) * P, (mb % 4 + 1) * P)
            ps = kqvps.tile([P, 512], F32, tag="kqv", name="ps_v")
            for dc in range(DC):
                nc.tensor.matmul(
                    ps[:, :HPC * HD], lhsT=xtb_q[ns][:, dc, msl],
                    rhs=wv[:, dc, :],
                    start=(dc == 0), stop=(dc == DC - 1),
                )
            nc.vector.tensor_tensor(
                out=v[:].rearrange("p m (h c) -> p m h c", c=HD + 1)[:, mb, :, 0:HD],
                in0=ps[:, :HPC * HD].rearrange("p (h e) -> p h e", e=HD),
                in1=vbias[:].rearrange("p (h e) -> p h e", e=HD),
                op=mybir.AluOpType.add,
            )

        def kqv_units(ns, mbs=None, kq=True):
            units = []
            if kq:
                for pr in range(2):
                    for which in range(2):
                        units.append(lambda ns=ns, pr=pr, which=which:
                                     emit_kq_unit(ns, pr, which))
            if mbs is None:
                mbs = range(4 * ns, 4 * ns + 4)
            for mb in mbs:
                units.append(lambda ns=ns, mb=mb: emit_v_unit(ns, mb))
            return units

        # filler machinery: KQV work units dripped into the attention stream
        filler_q = []
        fill_stat = {"queued": 0, "popped": 0}
        tick_n = [1]

        def enqueue_fillers(units):
            filler_q.extend(units)
            fill_stat["queued"] += len(units)
            return fill_stat["queued"]

        def round_tick():
            for _ in range(tick_n[0]):
                if filler_q:
                    filler_q.pop(0)()
                    fill_stat["popped"] += 1

        def flush_through(target):
            while fill_stat["popped"] < target and filler_q:
                filler_q.pop(0)()
                fill_stat["popped"] += 1

        def flush_fillers():
            while filler_q:
                filler_q.pop(0)()
                fill_stat["popped"] += 1

        def emit_attention_block(J):
            """Two heads of each partition-pair in lockstep; PV lags the
            S/exp pipeline so ACT overlaps PE."""
            nsl = slice(J * NB, (J + 1) * NB)
            n_mb = 2 * (J + 1)
            for pr in range(2):
                heads = []
                for h2 in range(2):
                    # one PSUM bank per head; den row at partition 64. Late
                    # blocks borrow the idle kqv pool for pr=1 so the second
                    # pair never waits on the first pair's bank release.
                    if J >= 6 and pr == 1:
                        bank = kqvps.tile([P, 512], F32, tag="kqv",
                                          name=f"ps_acc{h2}")
                    else:
                        bank = acc_ps.tile([P, 512], F32, tag="acc",
                                           name=f"ps_acc{h2}")
                    heads.append({"h2": h2, "l": 2 * pr + h2,
                                  "prow": slice(HD * h2, HD * h2 + HD),
                                  "opsf": bank[:, 0:NB]})
                pending = []  # (head, c0, cn, pts)

                def emit_s(hd, c0, cn):
                    # the last chunk holds the diagonal pair; block 2J+1
                    # only computes (and exps) its valid upper n-half
                    has_diag = c0 <= 2 * J < c0 + cn
                    wid = cn * NB - (128 if has_diag else 0)
                    sps = strip_ps.tile(
                        [P, CHUNK * NB], F32, tag="strip", name="ps_strip"
                    )[:, :wid]
                    for a in range(c0, c0 + cn):
                        o = (a - c0) * NB
                        if has_diag and a == 2 * J + 1:
                            nc.tensor.matmul(
                                sps[:, o:o + 128],
                                lhsT=q2[hd["prow"], pr, a * P:(a + 1) * P],
                                rhs=k2[hd["prow"], pr,
                                       J * NB + 128:(J + 1) * NB],
                                start=True, stop=True,
                            )
                        else:
                            nc.tensor.matmul(
                                sps[:, o:o + NB],
                                lhsT=q2[hd["prow"], pr, a * P:(a + 1) * P],
                                rhs=k2[hd["prow"], pr, nsl],
                                start=True, stop=True,
                            )
                    pts = pt_pool.tile(
                        [P, CHUNK * NB], BF16, tag="pt", name="pt"
                    )[:, :wid]
                    nc.scalar.activation(
                        pts, sps, mybir.ActivationFunctionType.Exp,
                        scale=1.0 / np.sqrt(HD),
                    )
                    if has_diag:
                        o = (2 * J - c0) * NB
                        nc.vector.tensor_tensor(
                            out=pts[:, o:o + 384], in0=pts[:, o:o + 384],
                            in1=mask[:], op=mybir.AluOpType.mult,
                        )
                    pending.append((hd, c0, cn, pts, has_diag))

                def emit_pv(hd, c0, cn, pts, has_diag):
                    lcol = hd["l"] * (HD + 1)
                    for a in range(c0, c0 + cn):
                        o = (a - c0) * NB
                        if has_diag and a == 2 * J + 1:
                            nc.tensor.matmul(
                                hd["opsf"][0:HD + 1, 128:NB],
                                lhsT=v[:, a, lcol:lcol + HD + 1],
                                rhs=pts[:, o:o + 128],
                                start=False, stop=(a == n_mb - 1),
                            )
                        else:
                            nc.tensor.matmul(
                                hd["opsf"][0:HD + 1, :],
                                lhsT=v[:, a, lcol:lcol + HD + 1],
                                rhs=pts[:, o:o + NB],
                                start=(a == 0), stop=(a == n_mb - 1),
                            )

                first = True
                for c0 in range(0, n_mb, CHUNK):
                    cn = min(CHUNK, n_mb - c0)
                    if not first:
                        round_tick()
                    emit_s(heads[0], c0, cn)
                    emit_s(heads[1], c0, cn)
                    while len(pending) > 7:
                        emit_pv(*pending.pop(0))
                    first = False
                while pending:
                    emit_pv(*pending.pop(0))

                # finalize: rc = 1/den, gpsimd broadcast to 64 partitions
                # (SBUF), one normalize-mult PSUM x SBUF -> saT bf16.
                for hd in heads:
                    round_tick()
                    h2 = hd["h2"]
                    rc = small.tile([1, NB], F32, tag="rc", name="rc")
                    nc.vector.reciprocal(rc[:], hd["opsf"][HD:HD + 1, :])
                    bc = bc_pool.tile([HD, NB], F32, tag="bc", name="bc")
                    nc.gpsimd.partition_broadcast(bc[:], rc[:], channels=HD)
                    nc.vector.tensor_tensor(
                        out=saT[hd["prow"], pr, nsl],
                        in0=hd["opsf"][0:HD, :],
                        in1=bc[:],
                        op=mybir.AluOpType.mult,
                    )

        def emit_a2a_stage(k):
            # t=0 (pr0, finalizes first) and t=1 on separate engine queues
            # so the two staging DMAs overlap
            nsl = slice(RBASE[k], RBASE[k] + RANGES[k])
            for t, eng in ((0, nc.sync), (1, nc.sync)):
                eng.dma_start(
                    cc_in[k][:, t].rearrange("s p r -> p s r"),
                    saT[:, t, nsl].rearrange("p (s r) -> p s r", r=RSUB[k]),
                )

        def emit_a2a(k):
            # 2-D receiver-major APs: byte-contiguous (HW requirement)
            # with the per-receiver chunk dim explicit
            nc.gpsimd.collective_compute(
                "AllToAll", mybir.AluOpType.bypass,
                replica_groups=GROUP8,
                ins=[cc_in_f[k][:].rearrange("s t p r -> s (t p r)")],
                outs=[cc_out_f[k][:].rearrange("s t p r -> s (t p r)")],
            )

        proj_mms = {}

        def proj_units(k):
            # saTg[p, fc, rcol]: fc = gs*2 + t (feature chunk),
            # rcol = bs*sub + r over both batches = 2*sub columns
            sub = RSUB[k]
            saTg = saTg_pool.tile([P, DC, 2 * max(RSUB)], BF16, tag="saTg",
                                  name="saTg")[:, :, :2 * sub]

            def load_unit():
                for bs in range(2):
                    nc.sync.dma_start(
                        saTg[:, :, bs * sub:(bs + 1) * sub].rearrange(
                            "p (gs t) r -> p gs t r", t=2),
                        cc_out[k][4 * bs:4 * bs + 4, :].rearrange(
                            "gs t p r -> p gs t r"),
                    )

            def mm_unit(s, half):
                rows = min(P, 2 * sub - s * P)
                pps = kqvps.tile([P, 512], F32, tag="kqv", name="ps_proj")
                hsl = slice(half * 512, (half + 1) * 512)
                for f in range(DC):
                    nc.tensor.matmul(
                        pps[:rows],
                        lhsT=saTg[:, f, s * P:s * P + rows],
                        rhs=wpt[:, f, hsl],
                        start=(f == 0), stop=(f == DC - 1),
                    )
                ost = ost_pool.tile([P, 512], F32, tag="ost", name="ost")
                nc.vector.tensor_tensor(
                    out=ost[:rows], in0=pps[:rows],
                    in1=bproj[:rows, hsl], op=mybir.AluOpType.add,
                )
                nc.sync.dma_start(
                    out[OBASE[k] + s * P:OBASE[k] + s * P + rows, hsl],
                    ost[:rows],
                )

            def mm_unit_q(s, qtr):
                # 256-col accumulation group: shorter bias->out tail chain
                pps = kqvps.tile([P, 512], F32, tag="kqv", name="ps_projq")
                rows = min(P, 2 * sub - s * P)
                qsl = slice(qtr * NB, (qtr + 1) * NB)
                for f in range(DC):
                    nc.tensor.matmul(
                        pps[:rows, 0:NB],
                        lhsT=saTg[:, f, s * P:s * P + rows],
                        rhs=wpt[:, f, qsl],
                        start=(f == 0), stop=(f == DC - 1),
                    )
                ost = ost_pool.tile([P, 512], F32, tag="ost",
                                    name="ost")[:, 0:NB]
                nc.vector.tensor_tensor(
                    out=ost[:rows], in0=pps[:rows, 0:NB],
                    in1=bproj[:rows, qsl], op=mybir.AluOpType.add,
                )
                nc.sync.dma_start(
                    out[OBASE[k] + s * P:OBASE[k] + s * P + rows, qsl],
                    ost[:rows],
                )

            units = []
            n_strip = (2 * sub + P - 1) // P
            for s in range(n_strip):
                if k == len(RANGES) - 1:
                    for qtr in range(4):
                        units.append(lambda s=s, qtr=qtr: mm_unit_q(s, qtr))
                else:
                    for half in range(2):
                        units.append(lambda s=s, half=half: mm_unit(s, half))
            return load_unit, units

        def emit_proj_load(k):
            load, mms = proj_units(k)
            load()
            proj_mms[k] = mms

        def proj_filler_units(k):
            return proj_mms.pop(k)

        def emit_proj_mms(k):
            for u in proj_mms.pop(k):
                u()

        # ---------------- emission order ----------------
        emit_consts_a()
        emit_warm(24, 36)
        emit_x_load(0, split=True)
        emit_consts_a2()
        emit_consts_b()
        emit_x_load(1)
        emit_x_load(2)
        emit_x_load(3)
        for u in kqv_units(0):
            u()
        emit_consts_c()
        mark1 = enqueue_fillers(kqv_units(1))
        tick_n[0] = 2
        emit_attention_block(0)
        mark2 = enqueue_fillers(kqv_units(2))
        emit_attention_block(1)
        tick_n[0] = 1
        flush_through(mark1)     # kqv(1) must complete before attn(2)
        emit_attention_block(2)
        mark3 = enqueue_fillers(kqv_units(3, mbs=[12, 13]))
        emit_attention_block(3)
        flush_through(mark2)     # kqv(2) must complete before attn(4)
        emit_wpt()
        emit_attention_block(4)
        emit_attention_block(5)
        flush_through(mark3)     # KQ(3)+V(12,13) must precede attn(6)
        emit_a2a_stage(0)
        emit_a2a(0)              # rows 0:1536 overlap attn(6)+attn(7)
        enqueue_fillers(kqv_units(3, mbs=[14, 15], kq=False))
        emit_attention_block(6)
        flush_fillers()          # V(14,15) must precede attn(7)
        emit_attention_block(7)
        # stage+fire a2a(1) FIRST: any load emitted earlier would hold the
        # SP sequencer through its wait and delay this staging
        emit_a2a_stage(1)
        emit_a2a(1)              # rows 1536:2048
        # bridge the a2a(0) spill window so proj(0) dispatches warm,
        # then proj(0) + a second bridge fill the a2a(1) window
        emit_warm(0, 26, bcols=512)
        emit_proj_load(0)
        emit_proj_mms(0)
        emit_warm(0, 40, bcols=512)
        emit_proj_load(1)
        emit_proj_mms(1)


def build_nc():
    nc = bacc.Bacc(
        "TRN2", target_bir_lowering=False, debug=False,
        num_devices=N_CORES, enable_asserts=False,
    )
    with tile.TileContext(nc) as tc:
        import contextlib
        with contextlib.ExitStack() as ctx:
            build_kernel(tc, ctx)
    nc.finalize()
    return nc


def make_in_maps(x, W_kqv, b_kqv, W_proj, b_proj):
    """Host-side shard + bf16 cast + layout packing (no math beyond rounding)."""
    in_maps = []
    # wpt[p, f, j] = W_proj[j, f*128+p]
    wpt = np.ascontiguousarray(
        np.asarray(W_proj, np.float32).T.reshape(DC, P, D).transpose(1, 0, 2)
    ).astype(BF16_NP)
    bp_rep = np.ascontiguousarray(
        np.broadcast_to(np.asarray(b_proj, np.float32)[None, :], (P, D)))
    for c in range(N_CORES):
        b = c // 4
        g = c % 4
        wl = np.ascontiguousarray(W_kqv[4 * g:4 * g + 4], np.float32)
        bl = np.ascontiguousarray(b_kqv[4 * g:4 * g + 4], np.float32)
        # [p, l, dc, e] view of the per-head weights (l = local head)
        wr = wl.reshape(HPC, DC, P, 3 * HD).transpose(2, 0, 1, 3)
        # wk2/wq2: [p, pr, dc, h2*64+e]
        wk2 = np.ascontiguousarray(
            wr[:, :, :, 0:HD].reshape(P, 2, 2, DC, HD).transpose(0, 1, 3, 2, 4)
            .reshape(P, 2, DC, P)).astype(BF16_NP)
        wq2 = np.ascontiguousarray(
            wr[:, :, :, HD:2 * HD].reshape(P, 2, 2, DC, HD).transpose(0, 1, 3, 2, 4)
            .reshape(P, 2, DC, P)).astype(BF16_NP)
        # wv: [p, dc, l*64+e]
        wv = np.ascontiguousarray(
            wr[:, :, :, 2 * HD:3 * HD].transpose(0, 2, 1, 3)
            .reshape(P, DC, HPC * HD)).astype(BF16_NP)
        # bkq: [64*h2+e, pr, {k,q}]
        bkq = np.zeros((P, 2, 2), np.float32)
        for pr in range(2):
            for h2 in range(2):
                l = 2 * pr + h2
                bkq[64 * h2:64 * h2 + 64, pr, 0] = bl[l, 0:HD]
                bkq[64 * h2:64 * h2 + 64, pr, 1] = bl[l, HD:2 * HD]
        vbias = np.ascontiguousarray(
            np.broadcast_to(bl[:, 2 * HD:3 * HD].reshape(1, HPC * HD),
                            (P, HPC * HD)))
        # xT: [p, dc, n] = x[b, n, dc*128+p]
        xt = np.ascontiguousarray(
            np.asarray(x[b], np.float32).T.reshape(DC, P, N).transpose(1, 0, 2))
        in_maps.append({
            "xtb": xt.astype(BF16_NP),
            "wk2": wk2,
            "wq2": wq2,
            "wv": wv,
            "bkq": bkq,
            "vbias": vbias,
            "bproj": bp_rep,
            "wpt": wpt,
        })
    return in_maps


def assemble(results):
    full = np.zeros((2, N, D), dtype=np.float32)
    for c in range(N_CORES):
        o = results[c]["out"]
        for k in range(len(RANGES)):
            sub = RSUB[k]
            r0 = RBASE[k] + sub * c
            for b in range(2):
                full[b, r0:r0 + sub, :] = \
                    o[OBASE[k] + sub * b:OBASE[k] + sub * (b + 1), :]
    return full


def kernel(x, W_kqv, b_kqv, W_proj, b_proj):
    x = np.asarray(x)
    W_kqv = np.asarray(W_kqv)
    b_kqv = np.asarray(b_kqv)
    W_proj = np.asarray(W_proj)
    b_proj = np.asarray(b_proj)
    nc = build_nc()
    in_maps = make_in_maps(x, W_kqv, b_kqv, W_proj, b_proj)
    res = run_bass_kernel_spmd(nc, in_maps, list(range(N_CORES)))
    return assemble(res.results)


if __name__ == "__main__":
    rng = np.random.default_rng(0)
    x = rng.standard_normal((2, N, D), dtype=np.float32)
    W_kqv = rng.standard_normal((H, D, 3 * HD), dtype=np.float32) / 32
    b_kqv = rng.standard_normal((H, 3 * HD), dtype=np.float32) / 32
    W_proj = rng.standard_normal((D, D), dtype=np.float32) / 32
    b_proj = rng.standard_normal((D,), dtype=np.float32) / 32
    out = kernel(x, W_kqv, b_kqv, W_proj, b_proj)
    print(out.shape, out.dtype, np.abs(out).max())
